# revision 1
# baseline (speedup 1.0000x reference)
"""Causal self-attention kernel for 8 Trainium2 NeuronCores.

Problem: B=4, T=2048, C=1024, NH=16, HD=64 (fp32).
Sharding: 8 cores = 4 batches x 2 head-groups (8 heads each).
Each core computes qkv projection + causal attention + its partial c_proj
for (batch b, heads hg*8..hg*8+7); host sums the two head-group partials.

On-device dataflow (per core, all matmuls float32r):
  x[b] --PE-transpose--> x^T --> q^T,k^T in [feat, T] layout (head-pair
  packed: 2 heads x 64 dims = 128 partitions) and v in [T, feat] layout
  augmented with a ones column per head (softmax denominator trick).
  S^T[k,q] = k^T.T @ q^T via two row-packed K=64 matmuls (tile_position);
  causal mask added with an identity matmul; exp on ScalarE over the
  [128,1024] two-head PSUM span; y~^T = v_aug.T @ P^T accumulated on PE
  (row 64 = denominator). Normalize with DVE reciprocal_approx_fast +
  GPSIMD partition_broadcast, then c_proj from y^T tiles.

Phase emission order P0 P1 A0 P2 C0 A1 P3 C1 A2 C2 A3 C3 keeps the
in-order PE queue from stalling on cross-phase dependency chains
(c_proj(c) needs the attention-epilogue normalize of chunk c; emitting
it two phases later hides that latency and keeps HAM warm).
"""

import math

import numpy as np

import concourse.bass as bass
import concourse.mybir as mybir
import concourse.tile as tile
from concourse import bacc
from concourse.bass_utils import run_bass_kernel_spmd

F32R = mybir.dt.float32r
F32 = mybir.dt.float32
EXP = mybir.ActivationFunctionType.Exp

B, T, C = 4, 2048, 1024
NH, HD = 16, 64
NHL = 8            # heads per core
PAIRS = 4          # head pairs per core
CH = 512           # q-chunk width
NCH = T // CH      # 4 q-chunks
KT = C // 128      # 8 contraction tiles over C
NTT = T // 128     # 16 T-tiles
SCALE = 1.0 / math.sqrt(HD)
NEG = -1.0e30


def build_nc():
    nc = bacc.Bacc("TRN2", target_bir_lowering=False)

    x_d = nc.dram_tensor("x_l", [T, C], F32R, kind="ExternalInput")
    wqk_d = nc.dram_tensor("w_qk", [1024, 1024], F32R, kind="ExternalInput")
    wv_d = nc.dram_tensor("w_v", [128, 4096], F32R, kind="ExternalInput")
    wp_d = nc.dram_tensor("w_p", [128, 4096], F32R, kind="ExternalInput")
    bqk_d = nc.dram_tensor("b_qk", [128, 8], F32, kind="ExternalInput")
    bv_d = nc.dram_tensor("b_v", [512], F32, kind="ExternalInput")
    bo_d = nc.dram_tensor("b_o", [C], F32, kind="ExternalInput")
    id_d = nc.dram_tensor("ident", [128, 128], F32R, kind="ExternalInput")
    mask_d = nc.dram_tensor("masks", [128, 1280], F32R, kind="ExternalInput")
    out_d = nc.dram_tensor("out_p", [T, C], F32, kind="ExternalOutput")

    with tile.TileContext(nc) as tc:
        with tc.tile_pool(name="cp", bufs=1) as cp, \
             tc.tile_pool(name="wk", bufs=1) as wk, \
             tc.tile_pool(name="ps", bufs=1, space="PSUM") as ps:
            # ---- constants (ident first: first transposes need only it) ----
            ident = cp.tile([128, 128], F32R, name="ident")
            nc.scalar.dma_start(ident, id_d.ap())
            # prefetch chunk-0 x tiles before the bulky constants
            xin0 = []
            for t4 in range(4):
                xi = wk.tile([128, C], F32R, tag="xin", bufs=4,
                             name=f"xin{t4}")
                nc.sync.dma_start(xi, x_d.ap()[t4 * 128:(t4 + 1) * 128, :])
                xin0.append(xi)
            bqk = cp.tile([128, 8], F32, name="bqk")
            nc.scalar.dma_start(bqk, bqk_d.ap())
            wv = cp.tile([128, 8, 512], F32R, name="wv")
            bv_row = cp.tile([1, 512], F32, name="bv_row")
            bv_rep = cp.tile([128, 512], F32, name="bv_rep")
            masks = cp.tile([128, 1280], F32R, name="masks")
            mask_off = {0: 0, 1: 128, 2: 384, 3: 768}
            bo_row = cp.tile([1, 1024], F32, name="bo_row")
            bo_rep = cp.tile([128, 1024], F32, name="bo_rep")
            wp = cp.tile([128, 4, 2, 512], F32R, name="wp")
            consts_loaded = set()

            def load_v_consts():
                if "v" in consts_loaded:
                    return
                consts_loaded.add("v")
                nc.scalar.dma_start(
                    wv, wv_d.ap().rearrange("p (a n) -> p a n", n=512))
                nc.scalar.dma_start(
                    bv_row, bv_d.ap().rearrange("(a n) -> a n", a=1))
                nc.gpsimd.partition_broadcast(bv_rep, bv_row)

            def load_a_consts():
                if "a" in consts_loaded:
                    return
                consts_loaded.add("a")
                nc.scalar.dma_start(masks, mask_d.ap())

            def load_c_consts():
                if "c" in consts_loaded:
                    return
                consts_loaded.add("c")
                nc.scalar.dma_start(
                    wp, wp_d.ap().rearrange("p (a b n) -> p a b n",
                                            a=4, b=2, n=512))
                nc.scalar.dma_start(
                    bo_row, bo_d.ap().rearrange("(a n) -> a n", a=1))
                nc.gpsimd.partition_broadcast(bo_rep, bo_row)

            # ---- persistent activations ----
            kT = [cp.tile([128, T], F32R, name=f"kT{p}") for p in range(PAIRS)]
            vt = cp.tile([128, NTT, 8 * 65], F32R, name="vt")

            qT = {}   # (pair, chunk) -> [128, 512] tile
            yT = {}   # (pair, chunk) -> [128, 512] tile
            XTS = {}  # chunk -> list of x^T tiles

            def proj_xt(c, xin_pre=None):
                ctx = nc.named_scope(f"xt{c}"); ctx.__enter__()
                xin = []
                for t4 in range(4):
                    tt = c * 4 + t4
                    if xin_pre is not None:
                        xi = xin_pre[t4]
                    else:
                        xi = wk.tile([128, C], F32R, tag="xin", bufs=4,
                                     name=f"xin{tt}")
                        nc.sync.dma_start(xi,
                                          x_d.ap()[tt * 128:(tt + 1) * 128, :])
                    xin.append(xi)
                xts = []
                for kc in range(KT):
                    xt_ps = ps.tile([128, 512], F32R, tag="pj", bufs=2,
                                    name=f"xtps{c}_{kc}")
                    for t4 in range(4):
                        nc.tensor.transpose(
                            xt_ps[:, t4 * 128:(t4 + 1) * 128],
                            xin[t4][:, kc * 128:(kc + 1) * 128], ident)
                    xt = wk.tile([128, 512], F32R, tag="xt", bufs=8,
                                 name=f"xt{c}_{kc}")
                    nc.vector.tensor_copy(xt, xt_ps)
                    xts.append(xt)
                XTS[c] = xts
                ctx.__exit__(None, None, None)

            def proj_qk(c, half):
                ctx = nc.named_scope(f"qk{c}_{half}"); ctx.__enter__()
                xts = XTS[c]
                for f in range(4 * half, 4 * half + 4):
                    wq = wk.tile([128, 8, 128], F32R, tag="wqk", bufs=2,
                                 name=f"wq{c}_{f}")
                    nc.scalar.dma_start(
                        wq, wqk_d.ap()[f * 128:(f + 1) * 128, :]
                        .rearrange("p (a j) -> p a j", j=128))
                    qk_ps = ps.tile([128, 512], F32, tag="pj", bufs=2,
                                    name=f"qkps{c}_{f}")
                    for kt in range(KT):
                        nc.tensor.matmul(qk_ps, wq[:, kt, :], xts[kt],
                                         start=(kt == 0), stop=(kt == KT - 1))
                    if f < 4:
                        qt = wk.tile([128, 512], F32R, tag="qT", bufs=7,
                                     name=f"qT{f}_{c}")
                        nc.vector.tensor_scalar_add(qt, qk_ps, bqk[:, f:f + 1])
                        qT[(f, c)] = qt
                    else:
                        nc.vector.tensor_scalar_add(
                            kT[f - 4][:, c * CH:(c + 1) * CH], qk_ps,
                            bqk[:, f:f + 1])
                ctx.__exit__(None, None, None)

            def proj_v(c):
                ctx = nc.named_scope(f"v{c}"); ctx.__enter__()
                load_v_consts()
                xts = XTS[c]
                for t4 in range(4):
                    tt = c * 4 + t4
                    v_ps = ps.tile([128, 512], F32, tag="pj", bufs=2,
                                   name=f"vps{tt}")
                    for kt in range(KT):
                        nc.tensor.matmul(v_ps, xts[kt][:, t4 * 128:(t4 + 1) * 128],
                                         wv[:, kt, :],
                                         start=(kt == 0), stop=(kt == KT - 1))
                    vslice = vt[:, tt, :].rearrange("p (h e) -> p h e", e=65)
                    nc.gpsimd.memset(
                        vt[:, tt, :].bitcast(F32)
                        .rearrange("p (h e) -> p h e", e=65)[:, :, 64:65], 1.0)
                    nc.vector.tensor_add(
                        vslice[:, :, 0:64],
                        v_ps.rearrange("p (h e) -> p h e", e=64),
                        bv_rep.rearrange("p (h e) -> p h e", e=64))
                del XTS[c]
                ctx.__exit__(None, None, None)

            def attn_pair(c, p):
                ctx = nc.named_scope(f"at{c}_{p}"); ctx.__enter__()
                load_a_consts()
                nkt = 4 * (c + 1)
                yA = ps.tile([65, 512], F32, tag="y", bufs=2,
                             name=f"yA{p}_{c}")
                yB = ps.tile([65, 512], F32, tag="y", bufs=2,
                             name=f"yB{p}_{c}")
                qtc = qT.pop((p, c))
                for kt in range(nkt):
                    s_ps = ps.tile([128, 1024], F32, tag="s", bufs=2,
                                   name=f"s{p}_{c}_{kt}")
                    d = kt * 128 - c * CH
                    partial = d >= 0
                    ksl = kT[p][:, kt * 128:(kt + 1) * 128]
                    nc.tensor.matmul(s_ps[:, 0:512], ksl[0:64, :],
                                     qtc[0:64, :], start=True,
                                     stop=not partial, tile_position=(0, 0))
                    nc.tensor.matmul(s_ps[:, 512:1024], ksl[64:128, :],
                                     qtc[64:128, :], start=True,
                                     stop=not partial,
                                     tile_position=(64, 0))
                    if partial:
                        n = d + 128
                        mo = mask_off[d // 128]
                        msl = masks[:, mo:mo + n]
                        nc.tensor.matmul(s_ps[:, 0:n], ident, msl,
                                         start=False, stop=True)
                        nc.tensor.matmul(s_ps[:, 512:512 + n], ident, msl,
                                         start=False, stop=True)
                    pt = wk.tile([128, 1024], F32R, tag="P", bufs=3,
                                 name=f"P{p}_{c}_{kt}")
                    nc.scalar.activation(pt, s_ps, EXP, scale=SCALE)
                    nc.tensor.matmul(
                        yA, vt[:, kt, (2 * p) * 65:(2 * p) * 65 + 65],
                        pt[:, 0:512],
                        start=(kt == 0), stop=(kt == nkt - 1))
                    nc.tensor.matmul(
                        yB, vt[:, kt, (2 * p + 1) * 65:(2 * p + 1) * 65 + 65],
                        pt[:, 512:1024],
                        start=(kt == 0), stop=(kt == nkt - 1))
                yt = wk.tile([128, 512], F32R, tag="yT", bufs=8,
                             name=f"yT{p}_{c}")
                for h, yps in ((0, yA), (1, yB)):
                    drow = wk.tile([1, 512], F32, tag="rc", bufs=2,
                                   name=f"dr{p}_{c}_{h}")
                    nc.vector.tensor_copy(drow, yps[64:65, :])
                    rc = wk.tile([1, 512], F32, tag="rc", bufs=2,
                                 name=f"rc{p}_{c}_{h}")
                    nc.vector.reciprocal_approx_fast(rc, drow)
                    rr = wk.tile([64, 512], F32, tag="rr", bufs=1,
                                 name=f"rr{p}_{c}_{h}")
                    nc.gpsimd.partition_broadcast(rr, rc)
                    nc.vector.tensor_mul(yt[h * 64:(h + 1) * 64, :],
                                         yps[0:64, :], rr)
                yT[(p, c)] = yt
                ctx.__exit__(None, None, None)

            def cproj_half(c, half):
                ctx = nc.named_scope(f"cp{c}_{half}"); ctx.__enter__()
                load_c_consts()
                for t4 in range(2 * half, 2 * half + 2):
                    tt = c * 4 + t4
                    for oc in range(2):
                        o_ps = ps.tile([128, 512], F32, tag="pj", bufs=2,
                                       name=f"ops{tt}_{oc}")
                        for p in range(PAIRS):
                            nc.tensor.matmul(
                                o_ps,
                                yT[(p, c)][:, t4 * 128:(t4 + 1) * 128],
                                wp[:, p, oc, :],
                                start=(p == 0), stop=(p == PAIRS - 1))
                        ot = wk.tile([128, 512], F32, tag="o", bufs=2,
                                     name=f"o{tt}_{oc}")
                        nc.vector.tensor_add(
                            ot, o_ps, bo_rep[:, oc * 512:(oc + 1) * 512])
                        nc.sync.dma_start(
                            out_d.ap()[tt * 128:(tt + 1) * 128,
                                       oc * 512:(oc + 1) * 512], ot)
                if half == 1:
                    for p in range(PAIRS):
                        yT.pop((p, c))
                ctx.__exit__(None, None, None)

            # fine-grained interleave: attention pairs alternate with
            # projection / c_proj slices so the in-order PE queue always has
            # exp-independent matmul work between ACT-dependent ones.
            proj_xt(0, xin_pre=xin0)
            proj_qk(0, 0)
            proj_qk(0, 1)
            proj_v(0)
            proj_xt(1)
            attn_pair(0, 0)
            proj_qk(1, 0)
            attn_pair(0, 1)
            proj_qk(1, 1)
            attn_pair(0, 2)
            proj_v(1)
            attn_pair(0, 3)
            proj_xt(2)
            attn_pair(1, 0)
            proj_qk(2, 0)
            attn_pair(1, 1)
            proj_qk(2, 1)
            attn_pair(1, 2)
            proj_v(2)
            attn_pair(1, 3)
            cproj_half(0, 0)
            attn_pair(2, 0)
            cproj_half(0, 1)
            attn_pair(2, 1)
            proj_xt(3)
            attn_pair(2, 2)
            proj_qk(3, 0)
            attn_pair(2, 3)
            proj_qk(3, 1)
            proj_v(3)
            cproj_half(1, 0)
            attn_pair(3, 0)
            cproj_half(1, 1)
            attn_pair(3, 1)
            cproj_half(2, 0)
            attn_pair(3, 2)
            cproj_half(2, 1)
            attn_pair(3, 3)
            cproj_half(3, 0)
            cproj_half(3, 1)

    nc.compile()
    return nc


_NC_CACHE = []


def _get_nc():
    if not _NC_CACHE:
        _NC_CACHE.append(build_nc())
    return _NC_CACHE[0]


def _host_consts():
    ident = np.eye(128, dtype=np.float32)
    kk = np.arange(128, dtype=np.int64)[:, None]
    masks = np.zeros((128, 1280), dtype=np.float32)
    off = 0
    for di in range(4):
        n = di * 128 + 128
        qq = np.arange(n, dtype=np.int64)[None, :]
        masks[:, off:off + n] = np.where(qq < kk + di * 128, NEG, 0.0)
        off += n
    return ident, masks


def _make_in_maps(x, W_attn, b_attn, W_proj, b_proj):
    ident, masks = _host_consts()
    in_maps = []
    for core in range(8):
        b, hg = core // 2, core % 2
        sl = slice(hg * 512, (hg + 1) * 512)
        w_q = W_attn[:, 0:1024][:, sl]
        w_k = W_attn[:, 1024:2048][:, sl]
        w_v = W_attn[:, 2048:3072][:, sl]
        in_maps.append({
            "x_l": np.ascontiguousarray(x[b]),
            "w_qk": np.ascontiguousarray(
                np.concatenate([w_q, w_k], axis=1).reshape(8, 128, 8, 128)
                .transpose(2, 1, 0, 3).reshape(1024, 1024)),
            "w_v": np.ascontiguousarray(
                w_v.reshape(8, 128, 512).transpose(1, 0, 2).reshape(128, 4096)),
            "w_p": np.ascontiguousarray(
                W_proj[sl, :].reshape(4, 128, 2, 512).transpose(1, 0, 2, 3)
                .reshape(128, 4096)),
            "b_qk": np.ascontiguousarray(
                np.concatenate([b_attn[0:1024][sl], b_attn[1024:2048][sl]])
                .reshape(8, 128).T),
            "b_v": np.ascontiguousarray(b_attn[2048:3072][sl]),
            "b_o": (b_proj if hg == 0
                    else np.zeros_like(b_proj)).astype(np.float32),
            "ident": ident,
            "masks": masks,
        })
    return in_maps


def _run(inputs, trace=False):
    x = np.asarray(inputs["x"], dtype=np.float32)
    W_attn = np.asarray(inputs["W_attn"], dtype=np.float32)
    b_attn = np.asarray(inputs["b_attn"], dtype=np.float32)
    W_proj = np.asarray(inputs["W_proj"], dtype=np.float32)
    b_proj = np.asarray(inputs["b_proj"], dtype=np.float32)

    nc = _get_nc()
    in_maps = _make_in_maps(x, W_attn, b_attn, W_proj, b_proj)
    res = run_bass_kernel_spmd(nc, in_maps, core_ids=list(range(8)),
                               trace=trace)
    out = np.empty((B, T, C), dtype=np.float32)
    for b in range(B):
        out[b] = res.results[2 * b]["out_p"] + res.results[2 * b + 1]["out_p"]
    return out, res


def kernel(**inputs) -> np.ndarray:
    out, _ = _run(inputs, trace=False)
    return out



# revision 2
# speedup vs baseline: 1.2654x; 1.2654x over previous
"""Causal self-attention kernel for 8 Trainium2 NeuronCores.

Problem: B=4, T=2048, C=1024, NH=16, HD=64 (fp32 in/out).
Sharding: 8 cores = 4 batches x 2 head-groups (8 heads each).
Each core computes qkv projection + causal attention + its partial c_proj
for (batch b, heads hg*8..hg*8+7); host sums the two head-group partials.

v2 (this file) vs the fp32r baseline:
  * x is transposed on the HOST and shipped as bf16 [C,T] — the PE
    transpose phase (128 transposes + 32 PSUM->SBUF copies) is gone.
  * all matmul operands are bf16 (tolerance is 2e-2; measured bf16
    pipeline error ~4e-3).  fp32r already streams 1 cyc/row at N>=256,
    but bf16 keeps 1 cyc/row at the N=128 tiles the trimmed diagonal
    introduces, halves DMA/SBUF, and doubles DVE throughput on 16-bit.
  * causal trimming: for the 4 diagonal k-tiles of each q-chunk the
    fully-masked q-range [0,d) is neither matmul'd, exp'd, nor fed to
    the PV matmul; only the 128-wide true-diagonal block gets a mask
    add (one shared [128,128] triangular mask, via identity matmul).
    Cuts S/exp/PV work in the diagonal region ~45% and the mask adds
    ~70% vs the baseline's full-width mask strips.

On-device dataflow per core (heads processed as 4 pairs of 2):
  q^T,k^T in [feat,T] layout (pair-packed: 2x64 dims = 128 partitions)
  from W^T x^T matmuls; v in [T,feat] layout augmented with a ones
  column per head (softmax denominator rides row 64 of the PV psum).
  S^T[k,q] via two row-packed K=64 matmuls (tile_position); exp on
  ScalarE over the two-head PSUM span; y~^T = v_aug.T @ P^T on PE.
  Normalize with DVE reciprocal + GPSIMD partition_broadcast, then
  c_proj from y^T tiles.  Phase emission interleaves attention pairs
  with projection/c_proj slices so the in-order PE queue always has
  exp-independent matmul work between ACT-dependent ones.
"""

import math

import numpy as np

import concourse.bass as bass
import concourse.mybir as mybir
import concourse.tile as tile
from concourse import bacc
from concourse.bass_utils import run_bass_kernel_spmd

BF16 = mybir.dt.bfloat16
F32 = mybir.dt.float32
EXP = mybir.ActivationFunctionType.Exp
BF16NP = np.dtype(mybir.dt.np(mybir.dt.bfloat16))

B, T, C = 4, 2048, 1024
NH, HD = 16, 64
NHL = 8            # heads per core
PAIRS = 4          # head pairs per core
CH = 512           # q-chunk width
NCH = T // CH      # 4 q-chunks
KT = C // 128      # 8 contraction tiles over C
NTT = T // 128     # 16 T-tiles
SCALE = 1.0 / math.sqrt(HD)
NEG = -1.0e30


def build_nc():
    nc = bacc.Bacc("TRN2", target_bir_lowering=False)

    xt_d = nc.dram_tensor("x_t", [C, T], BF16, kind="ExternalInput")
    wqk_d = nc.dram_tensor("w_qk", [1024, 1024], BF16, kind="ExternalInput")
    wv_d = nc.dram_tensor("w_v", [128, 4096], BF16, kind="ExternalInput")
    wp_d = nc.dram_tensor("w_p", [128, 4096], BF16, kind="ExternalInput")
    bqk_d = nc.dram_tensor("b_qk", [128, 8], F32, kind="ExternalInput")
    bv_d = nc.dram_tensor("b_v", [512], F32, kind="ExternalInput")
    bo_d = nc.dram_tensor("b_o", [C], F32, kind="ExternalInput")
    id_d = nc.dram_tensor("ident", [128, 128], BF16, kind="ExternalInput")
    mask_d = nc.dram_tensor("masks", [128, 128], BF16, kind="ExternalInput")
    out_d = nc.dram_tensor("out_p", [T, C], F32, kind="ExternalOutput")

    with tile.TileContext(nc) as tc:
        with tc.tile_pool(name="cp", bufs=1) as cp, \
             tc.tile_pool(name="wk", bufs=1) as wk, \
             tc.tile_pool(name="ps", bufs=1, space="PSUM") as ps:
            # ---- x^T resident tiles; chunk-0 slices land first ----
            xts = [cp.tile([128, T], BF16, name=f"xt{kt}") for kt in range(KT)]
            for c in range(NCH):
                for kt in range(KT):
                    nc.sync.dma_start(
                        xts[kt][:, c * CH:(c + 1) * CH],
                        xt_d.ap()[kt * 128:(kt + 1) * 128, c * CH:(c + 1) * CH])
            ident = cp.tile([128, 128], BF16, name="ident")
            nc.scalar.dma_start(ident, id_d.ap())
            mask = cp.tile([128, 128], BF16, name="mask")
            bqk = cp.tile([128, 8], F32, name="bqk")
            nc.scalar.dma_start(bqk, bqk_d.ap())
            wv = cp.tile([128, 8, 512], BF16, name="wv")
            bv_row = cp.tile([1, 512], F32, name="bv_row")
            bv_rep = cp.tile([128, 512], F32, name="bv_rep")
            bo_row = cp.tile([1, 1024], F32, name="bo_row")
            bo_rep = cp.tile([128, 1024], F32, name="bo_rep")
            wp = cp.tile([128, 4, 2, 512], BF16, name="wp")
            consts_loaded = set()

            def load_v_consts():
                if "v" in consts_loaded:
                    return
                consts_loaded.add("v")
                nc.scalar.dma_start(
                    wv, wv_d.ap().rearrange("p (a n) -> p a n", n=512))
                nc.scalar.dma_start(
                    bv_row, bv_d.ap().rearrange("(a n) -> a n", a=1))
                nc.gpsimd.partition_broadcast(bv_rep, bv_row)
                # ones plane for the softmax-denominator columns of v
                nc.gpsimd.memset(
                    vt.rearrange("p t (h e) -> p t h e", e=65)[:, :, :, 64:65],
                    1.0)

            def load_a_consts():
                if "a" in consts_loaded:
                    return
                consts_loaded.add("a")
                nc.scalar.dma_start(mask, mask_d.ap())

            def load_c_consts():
                if "c" in consts_loaded:
                    return
                consts_loaded.add("c")
                nc.scalar.dma_start(
                    wp, wp_d.ap().rearrange("p (a b n) -> p a b n",
                                            a=4, b=2, n=512))
                nc.scalar.dma_start(
                    bo_row, bo_d.ap().rearrange("(a n) -> a n", a=1))
                nc.gpsimd.partition_broadcast(bo_rep, bo_row)

            # ---- persistent activations ----
            kT = [cp.tile([128, T], BF16, name=f"kT{p}") for p in range(PAIRS)]
            vt = cp.tile([128, NTT, 8 * 65], BF16, name="vt")

            qT = {}   # (pair, chunk) -> [128, 512] tile
            yT = {}   # (pair, chunk) -> [128, 512] tile

            def proj_qk(c, half):
                ctx = nc.named_scope(f"qk{c}_{half}"); ctx.__enter__()
                for f in range(4 * half, 4 * half + 4):
                    wq = wk.tile([128, 8, 128], BF16, tag="wqk", bufs=2,
                                 name=f"wq{c}_{f}")
                    nc.scalar.dma_start(
                        wq, wqk_d.ap()[f * 128:(f + 1) * 128, :]
                        .rearrange("p (a j) -> p a j", j=128))
                    qk_ps = ps.tile([128, 512], F32, tag="pj", bufs=2,
                                    name=f"qkps{c}_{f}")
                    for kt in range(KT):
                        nc.tensor.matmul(qk_ps, wq[:, kt, :],
                                         xts[kt][:, c * CH:(c + 1) * CH],
                                         start=(kt == 0), stop=(kt == KT - 1))
                    if f < 4:
                        qt = wk.tile([128, 512], BF16, tag="qT", bufs=7,
                                     name=f"qT{f}_{c}")
                        nc.vector.tensor_scalar_add(qt, qk_ps, bqk[:, f:f + 1])
                        qT[(f, c)] = qt
                    else:
                        nc.vector.tensor_scalar_add(
                            kT[f - 4][:, c * CH:(c + 1) * CH], qk_ps,
                            bqk[:, f:f + 1])
                ctx.__exit__(None, None, None)

            def proj_v(c):
                ctx = nc.named_scope(f"v{c}"); ctx.__enter__()
                load_v_consts()
                for t4 in range(4):
                    tt = c * 4 + t4
                    v_ps = ps.tile([128, 512], F32, tag="pj", bufs=2,
                                   name=f"vps{tt}")
                    for kt in range(KT):
                        nc.tensor.matmul(v_ps,
                                         xts[kt][:, tt * 128:(tt + 1) * 128],
                                         wv[:, kt, :],
                                         start=(kt == 0), stop=(kt == KT - 1))
                    vslice = vt[:, tt, :].rearrange("p (h e) -> p h e", e=65)
                    nc.vector.tensor_add(
                        vslice[:, :, 0:64],
                        v_ps.rearrange("p (h e) -> p h e", e=64),
                        bv_rep.rearrange("p (h e) -> p h e", e=64))
                ctx.__exit__(None, None, None)

            def attn_pair(c, p):
                ctx = nc.named_scope(f"at{c}_{p}"); ctx.__enter__()
                load_a_consts()
                nfull = 4 * c
                yA = ps.tile([65, 512], F32, tag="y", bufs=2,
                             name=f"yA{p}_{c}")
                yB = ps.tile([65, 512], F32, tag="y", bufs=2,
                             name=f"yB{p}_{c}")
                qtc = qT.pop((p, c))
                for kt in range(nfull + 4):
                    di = kt - nfull
                    d = 0 if di < 0 else di * 128
                    s_ps = ps.tile([128, 1024], F32, tag="s", bufs=2,
                                   name=f"s{p}_{c}_{kt}")
                    ksl = kT[p][:, kt * 128:(kt + 1) * 128]
                    if di < 0:
                        # off-diagonal: full width, no mask
                        nc.tensor.matmul(s_ps[:, 0:512], ksl[0:64, :],
                                         qtc[0:64, :], start=True, stop=True,
                                         tile_position=(0, 0))
                        nc.tensor.matmul(s_ps[:, 512:1024], ksl[64:128, :],
                                         qtc[64:128, :], start=True, stop=True,
                                         tile_position=(64, 0))
                    else:
                        # diagonal k-tile: q < d is fully masked (skipped);
                        # [d,d+128) is the triangular block (mask add);
                        # [d+128,512) is unmasked.
                        if d + 128 < 512:
                            nc.tensor.matmul(
                                s_ps[:, d + 128:512], ksl[0:64, :],
                                qtc[0:64, d + 128:512], start=True, stop=True,
                                tile_position=(0, 0))
                            nc.tensor.matmul(
                                s_ps[:, 512 + d + 128:1024], ksl[64:128, :],
                                qtc[64:128, d + 128:512], start=True,
                                stop=True, tile_position=(64, 0))
                        nc.tensor.matmul(
                            s_ps[:, d:d + 128], ksl[0:64, :],
                            qtc[0:64, d:d + 128], start=True, stop=False,
                            tile_position=(0, 0))
                        nc.tensor.matmul(
                            s_ps[:, 512 + d:512 + d + 128], ksl[64:128, :],
                            qtc[64:128, d:d + 128], start=True, stop=False,
                            tile_position=(64, 0))
                        nc.tensor.matmul(s_ps[:, d:d + 128], ident, mask,
                                         start=False, stop=True)
                        nc.tensor.matmul(s_ps[:, 512 + d:512 + d + 128],
                                         ident, mask, start=False, stop=True)
                    pt = wk.tile([128, 1024], BF16, tag="P", bufs=3,
                                 name=f"P{p}_{c}_{kt}")
                    if d == 0:
                        nc.scalar.activation(pt, s_ps, EXP, scale=SCALE)
                    else:
                        nc.scalar.activation(pt[:, d:512], s_ps[:, d:512],
                                             EXP, scale=SCALE)
                        nc.scalar.activation(pt[:, 512 + d:1024],
                                             s_ps[:, 512 + d:1024],
                                             EXP, scale=SCALE)
                    nc.tensor.matmul(
                        yA[:, d:512], vt[:, kt, (2 * p) * 65:(2 * p) * 65 + 65],
                        pt[:, d:512],
                        start=(kt == 0), stop=(di == 3),
                        skip_group_check=(di >= 0))
                    nc.tensor.matmul(
                        yB[:, d:512],
                        vt[:, kt, (2 * p + 1) * 65:(2 * p + 1) * 65 + 65],
                        pt[:, 512 + d:1024],
                        start=(kt == 0), stop=(di == 3),
                        skip_group_check=(di >= 0))
                yt = wk.tile([128, 512], BF16, tag="yT", bufs=8,
                             name=f"yT{p}_{c}")
                for h, yps in ((0, yA), (1, yB)):
                    drow = wk.tile([1, 512], F32, tag="rc", bufs=2,
                                   name=f"dr{p}_{c}_{h}")
                    nc.vector.tensor_copy(drow, yps[64:65, :])
                    rc = wk.tile([1, 512], F32, tag="rc", bufs=2,
                                 name=f"rc{p}_{c}_{h}")
                    nc.vector.reciprocal_approx_fast(rc, drow)
                    rr = wk.tile([64, 512], F32, tag="rr", bufs=1,
                                 name=f"rr{p}_{c}_{h}")
                    nc.gpsimd.partition_broadcast(rr, rc)
                    nc.vector.tensor_mul(yt[h * 64:(h + 1) * 64, :],
                                         yps[0:64, :], rr)
                yT[(p, c)] = yt
                ctx.__exit__(None, None, None)

            def cproj_half(c, half):
                ctx = nc.named_scope(f"cp{c}_{half}"); ctx.__enter__()
                load_c_consts()
                for t4 in range(2 * half, 2 * half + 2):
                    tt = c * 4 + t4
                    for oc in range(2):
                        o_ps = ps.tile([128, 512], F32, tag="pj", bufs=2,
                                       name=f"ops{tt}_{oc}")
                        for p in range(PAIRS):
                            nc.tensor.matmul(
                                o_ps,
                                yT[(p, c)][:, t4 * 128:(t4 + 1) * 128],
                                wp[:, p, oc, :],
                                start=(p == 0), stop=(p == PAIRS - 1))
                        ot = wk.tile([128, 512], F32, tag="o", bufs=2,
                                     name=f"o{tt}_{oc}")
                        nc.vector.tensor_add(
                            ot, o_ps, bo_rep[:, oc * 512:(oc + 1) * 512])
                        nc.sync.dma_start(
                            out_d.ap()[tt * 128:(tt + 1) * 128,
                                       oc * 512:(oc + 1) * 512], ot)
                if half == 1:
                    for p in range(PAIRS):
                        yT.pop((p, c))
                ctx.__exit__(None, None, None)

            # fine-grained interleave: attention pairs alternate with
            # projection / c_proj slices so the in-order PE queue always has
            # exp-independent matmul work between ACT-dependent ones.
            proj_qk(0, 0)
            proj_qk(0, 1)
            proj_v(0)
            attn_pair(0, 0)
            proj_qk(1, 0)
            attn_pair(0, 1)
            proj_qk(1, 1)
            attn_pair(0, 2)
            proj_v(1)
            attn_pair(0, 3)
            attn_pair(1, 0)
            proj_qk(2, 0)
            attn_pair(1, 1)
            proj_qk(2, 1)
            attn_pair(1, 2)
            proj_v(2)
            attn_pair(1, 3)
            cproj_half(0, 0)
            attn_pair(2, 0)
            cproj_half(0, 1)
            attn_pair(2, 1)
            attn_pair(2, 2)
            proj_qk(3, 0)
            attn_pair(2, 3)
            proj_qk(3, 1)
            proj_v(3)
            cproj_half(1, 0)
            attn_pair(3, 0)
            cproj_half(1, 1)
            attn_pair(3, 1)
            cproj_half(2, 0)
            attn_pair(3, 2)
            cproj_half(2, 1)
            attn_pair(3, 3)
            cproj_half(3, 0)
            cproj_half(3, 1)

    nc.compile()
    return nc


_NC_CACHE = []


def _get_nc():
    if not _NC_CACHE:
        _NC_CACHE.append(build_nc())
    return _NC_CACHE[0]


def _host_consts():
    ident = np.eye(128, dtype=np.float32)
    kk = np.arange(128, dtype=np.int64)[:, None]
    jj = np.arange(128, dtype=np.int64)[None, :]
    tri = np.where(jj < kk, NEG, 0.0).astype(np.float32)
    return ident, tri


def _make_in_maps(x, W_attn, b_attn, W_proj, b_proj):
    ident, tri = _host_consts()
    in_maps = []
    for core in range(8):
        b, hg = core // 2, core % 2
        sl = slice(hg * 512, (hg + 1) * 512)
        w_q = W_attn[:, 0:1024][:, sl]
        w_k = W_attn[:, 1024:2048][:, sl]
        w_v = W_attn[:, 2048:3072][:, sl]
        in_maps.append({
            "x_t": np.ascontiguousarray(x[b].T).astype(BF16NP),
            "w_qk": np.ascontiguousarray(
                np.concatenate([w_q, w_k], axis=1).reshape(8, 128, 8, 128)
                .transpose(2, 1, 0, 3).reshape(1024, 1024)).astype(BF16NP),
            "w_v": np.ascontiguousarray(
                w_v.reshape(8, 128, 512).transpose(1, 0, 2)
                .reshape(128, 4096)).astype(BF16NP),
            "w_p": np.ascontiguousarray(
                W_proj[sl, :].reshape(4, 128, 2, 512).transpose(1, 0, 2, 3)
                .reshape(128, 4096)).astype(BF16NP),
            "b_qk": np.ascontiguousarray(
                np.concatenate([b_attn[0:1024][sl], b_attn[1024:2048][sl]])
                .reshape(8, 128).T),
            "b_v": np.ascontiguousarray(b_attn[2048:3072][sl]),
            "b_o": (b_proj if hg == 0
                    else np.zeros_like(b_proj)).astype(np.float32),
            "ident": ident.astype(BF16NP),
            "masks": tri.astype(BF16NP),
        })
    return in_maps


def _run(inputs, trace=False):
    x = np.asarray(inputs["x"], dtype=np.float32)
    W_attn = np.asarray(inputs["W_attn"], dtype=np.float32)
    b_attn = np.asarray(inputs["b_attn"], dtype=np.float32)
    W_proj = np.asarray(inputs["W_proj"], dtype=np.float32)
    b_proj = np.asarray(inputs["b_proj"], dtype=np.float32)

    nc = _get_nc()
    in_maps = _make_in_maps(x, W_attn, b_attn, W_proj, b_proj)
    res = run_bass_kernel_spmd(nc, in_maps, core_ids=list(range(8)),
                               trace=trace)
    out = np.empty((B, T, C), dtype=np.float32)
    for b in range(B):
        out[b] = res.results[2 * b]["out_p"] + res.results[2 * b + 1]["out_p"]
    return out, res


def kernel(**inputs) -> np.ndarray:
    out, _ = _run(inputs, trace=False)
    return out


# revision 10
# speedup vs baseline: 1.3165x; 1.0405x over previous
"""Causal self-attention kernel for 8 Trainium2 NeuronCores.

Problem: B=4, T=2048, C=1024, NH=16, HD=64 (fp32 in/out).
Sharding: 8 cores = 4 batches x 2 head-groups (8 heads each).
Each core computes qkv projection + causal attention + its partial c_proj
for (batch b, heads hg*8..hg*8+7); host sums the two head-group partials.

v2 (this file) vs the fp32r baseline:
  * x is transposed on the HOST and shipped as bf16 [C,T] — the PE
    transpose phase (128 transposes + 32 PSUM->SBUF copies) is gone.
  * all matmul operands are bf16 (tolerance is 2e-2; measured bf16
    pipeline error ~4e-3).  fp32r already streams 1 cyc/row at N>=256,
    but bf16 keeps 1 cyc/row at the N=128 tiles the trimmed diagonal
    introduces, halves DMA/SBUF, and doubles DVE throughput on 16-bit.
  * causal trimming: for the 4 diagonal k-tiles of each q-chunk the
    fully-masked q-range [0,d) is neither matmul'd, exp'd, nor fed to
    the PV matmul; only the 128-wide true-diagonal block gets a mask
    add (one shared [128,128] triangular mask, via identity matmul).
    Cuts S/exp/PV work in the diagonal region ~45% and the mask adds
    ~70% vs the baseline's full-width mask strips.

On-device dataflow per core (heads processed as 4 pairs of 2):
  q^T,k^T in [feat,T] layout (pair-packed: 2x64 dims = 128 partitions)
  from W^T x^T matmuls; v in [T,feat] layout augmented with a ones
  column per head (softmax denominator rides row 64 of the PV psum).
  S^T[k,q] via two row-packed K=64 matmuls (tile_position); exp on
  ScalarE over the two-head PSUM span; y~^T = v_aug.T @ P^T on PE.
  Normalize with DVE reciprocal + GPSIMD partition_broadcast, then
  c_proj from y^T tiles.  Phase emission interleaves attention pairs
  with projection/c_proj slices so the in-order PE queue always has
  exp-independent matmul work between ACT-dependent ones.
"""

import math

import numpy as np

import concourse.bass as bass
import concourse.mybir as mybir
import concourse.tile as tile
from concourse import bacc
from concourse.bass_utils import run_bass_kernel_spmd

BF16 = mybir.dt.bfloat16
F32 = mybir.dt.float32
EXP = mybir.ActivationFunctionType.Exp
BF16NP = np.dtype(mybir.dt.np(mybir.dt.bfloat16))

B, T, C = 4, 2048, 1024
NH, HD = 16, 64
NHL = 8            # heads per core
PAIRS = 4          # head pairs per core
CH = 512           # q-chunk width
NCH = T // CH      # 4 q-chunks
KT = C // 128      # 8 contraction tiles over C
NTT = T // 128     # 16 T-tiles
SCALE = 1.0 / math.sqrt(HD)
NEG = -1.0e30


def build_nc():
    nc = bacc.Bacc("TRN2", target_bir_lowering=False)

    xt_d = nc.dram_tensor("x_t", [C, T], BF16, kind="ExternalInput")
    wqk_d = nc.dram_tensor("w_qk", [1024, 1024], BF16, kind="ExternalInput")
    wv_d = nc.dram_tensor("w_v", [128, 4096], BF16, kind="ExternalInput")
    wp_d = nc.dram_tensor("w_p", [128, 4096], BF16, kind="ExternalInput")
    bqk_d = nc.dram_tensor("b_qk", [128, 8], F32, kind="ExternalInput")
    bv_d = nc.dram_tensor("b_v", [512], F32, kind="ExternalInput")
    bo_d = nc.dram_tensor("b_o", [128, 8], F32, kind="ExternalInput")
    id_d = nc.dram_tensor("ident", [128, 128], BF16, kind="ExternalInput")
    mask_d = nc.dram_tensor("masks", [128, 128], BF16, kind="ExternalInput")
    out_d = nc.dram_tensor("out_p", [C, T], F32, kind="ExternalOutput")

    with tile.TileContext(nc) as tc:
        with tc.tile_pool(name="cp", bufs=1) as cp, \
             tc.tile_pool(name="wk", bufs=1) as wk, \
             tc.tile_pool(name="ps", bufs=1, space="PSUM") as ps:
            # ---- x^T resident tiles; chunk-0 slices land first, spread
            # across the two HWDGE queues + gpsimd SWDGE ----
            xts = [cp.tile([128, T], BF16, name=f"xt{kt}") for kt in range(KT)]
            for kt in range(KT):
                eng = nc.sync if kt % 2 == 0 else nc.gpsimd
                eng.dma_start(xts[kt][:, 0:CH],
                              xt_d.ap()[kt * 128:(kt + 1) * 128, 0:CH])
            # resident W_qk: one [128,8(kt),128] block per output f-block
            wqk_s = cp.tile([128, 8, 8, 128], BF16, name="wqk_s")
            for f in range(8):
                nc.scalar.dma_start(
                    wqk_s[:, f, :, :],
                    wqk_d.ap()[f * 128:(f + 1) * 128, :]
                    .rearrange("p (a j) -> p a j", j=128))
            ident = cp.tile([128, 128], BF16, name="ident")
            nc.sync.dma_start(ident, id_d.ap())
            mask = cp.tile([128, 128], BF16, name="mask")
            bqk = cp.tile([128, 8], F32, name="bqk")
            nc.sync.dma_start(bqk, bqk_d.ap())
            for kt in range(KT):
                eng = nc.sync if kt % 2 == 0 else nc.gpsimd
                eng.dma_start(xts[kt][:, CH:T],
                              xt_d.ap()[kt * 128:(kt + 1) * 128, CH:T])
            wv = cp.tile([128, 8, 512], BF16, name="wv")
            bv_row = cp.tile([1, 512], F32, name="bv_row")
            bv_rep = cp.tile([128, 512], F32, name="bv_rep")
            bo = cp.tile([128, 8], F32, name="bo")
            wp = cp.tile([128, 4, 8, 128], BF16, name="wp")
            consts_loaded = set()

            def load_v_consts():
                if "v" in consts_loaded:
                    return
                consts_loaded.add("v")
                nc.scalar.dma_start(
                    wv, wv_d.ap().rearrange("p (a n) -> p a n", n=512))
                nc.scalar.dma_start(
                    bv_row, bv_d.ap().rearrange("(a n) -> a n", a=1))
                nc.gpsimd.partition_broadcast(bv_rep, bv_row)
                # ones plane for the softmax-denominator columns of v
                nc.gpsimd.memset(
                    vt.rearrange("p t (h e) -> p t h e", e=65)[:, :, :, 64:65],
                    1.0)

            def load_a_consts():
                if "a" in consts_loaded:
                    return
                consts_loaded.add("a")
                nc.scalar.dma_start(mask, mask_d.ap())

            def load_c_consts():
                if "c" in consts_loaded:
                    return
                consts_loaded.add("c")
                nc.scalar.dma_start(
                    wp, wp_d.ap().rearrange("p (a b n) -> p a b n",
                                            a=4, b=8, n=128))
                nc.scalar.dma_start(bo, bo_d.ap())

            # ---- persistent activations ----
            kT = [cp.tile([128, T], BF16, name=f"kT{p}") for p in range(PAIRS)]
            vt = cp.tile([128, NTT, 8 * 65], BF16, name="vt")

            qT = {}   # (pair, chunk) -> [128, 512] tile
            yT = {}   # (pair, chunk) -> [128, 512] tile

            def proj_qk(c, half):
                ctx = nc.named_scope(f"qk{c}_{half}"); ctx.__enter__()
                for f in range(4 * half, 4 * half + 4):
                    qk_ps = ps.tile([128, 512], F32, tag="pj", bufs=2,
                                    name=f"qkps{c}_{f}")
                    for kt in range(KT):
                        nc.tensor.matmul(qk_ps, wqk_s[:, f, kt, :],
                                         xts[kt][:, c * CH:(c + 1) * CH],
                                         start=(kt == 0), stop=(kt == KT - 1))
                    if f < 4:
                        qt = wk.tile([128, 512], BF16, tag="qT", bufs=7,
                                     name=f"qT{f}_{c}")
                        nc.vector.tensor_scalar_add(qt, qk_ps, bqk[:, f:f + 1])
                        qT[(f, c)] = qt
                    else:
                        nc.vector.tensor_scalar_add(
                            kT[f - 4][:, c * CH:(c + 1) * CH], qk_ps,
                            bqk[:, f:f + 1])
                ctx.__exit__(None, None, None)

            def proj_v(c):
                ctx = nc.named_scope(f"v{c}"); ctx.__enter__()
                load_v_consts()
                for t4 in range(4):
                    tt = c * 4 + t4
                    v_ps = ps.tile([128, 512], F32, tag="pj", bufs=2,
                                   name=f"vps{tt}")
                    for kt in range(KT):
                        nc.tensor.matmul(v_ps,
                                         xts[kt][:, tt * 128:(tt + 1) * 128],
                                         wv[:, kt, :],
                                         start=(kt == 0), stop=(kt == KT - 1))
                    vslice = vt[:, tt, :].rearrange("p (h e) -> p h e", e=65)
                    nc.vector.tensor_add(
                        vslice[:, :, 0:64],
                        v_ps.rearrange("p (h e) -> p h e", e=64),
                        bv_rep.rearrange("p (h e) -> p h e", e=64))
                ctx.__exit__(None, None, None)

            def attn_pair(c, p):
                ctx = nc.named_scope(f"at{c}_{p}"); ctx.__enter__()
                load_a_consts()
                nfull = 4 * c
                yA = ps.tile([65, 512], F32, tag="y", bufs=2,
                             name=f"yA{p}_{c}")
                yB = ps.tile([65, 512], F32, tag="y", bufs=2,
                             name=f"yB{p}_{c}")
                qtc = qT.pop((p, c))
                for kt in range(nfull + 4):
                    di = kt - nfull
                    d = 0 if di < 0 else di * 128
                    s_ps = ps.tile([128, 1024], F32, tag="s", bufs=2,
                                   name=f"s{p}_{c}_{kt}")
                    ksl = kT[p][:, kt * 128:(kt + 1) * 128]
                    if di < 0:
                        # off-diagonal: full width, no mask
                        nc.tensor.matmul(s_ps[:, 0:512], ksl[0:64, :],
                                         qtc[0:64, :], start=True, stop=True,
                                         tile_position=(0, 0))
                        nc.tensor.matmul(s_ps[:, 512:1024], ksl[64:128, :],
                                         qtc[64:128, :], start=True, stop=True,
                                         tile_position=(64, 0))
                    else:
                        # diagonal k-tile: q < d is fully masked (skipped);
                        # [d,d+128) is the triangular block (mask add);
                        # [d+128,512) is unmasked.
                        if d + 128 < 512:
                            nc.tensor.matmul(
                                s_ps[:, d + 128:512], ksl[0:64, :],
                                qtc[0:64, d + 128:512], start=True, stop=True,
                                tile_position=(0, 0))
                            nc.tensor.matmul(
                                s_ps[:, 512 + d + 128:1024], ksl[64:128, :],
                                qtc[64:128, d + 128:512], start=True,
                                stop=True, tile_position=(64, 0))
                        nc.tensor.matmul(
                            s_ps[:, d:d + 128], ksl[0:64, :],
                            qtc[0:64, d:d + 128], start=True, stop=False,
                            tile_position=(0, 0))
                        nc.tensor.matmul(
                            s_ps[:, 512 + d:512 + d + 128], ksl[64:128, :],
                            qtc[64:128, d:d + 128], start=True, stop=False,
                            tile_position=(64, 0))
                        nc.tensor.matmul(s_ps[:, d:d + 128], ident, mask,
                                         start=False, stop=True)
                        nc.tensor.matmul(s_ps[:, 512 + d:512 + d + 128],
                                         ident, mask, start=False, stop=True)
                    pt = wk.tile([128, 1024], BF16, tag="P", bufs=3,
                                 name=f"P{p}_{c}_{kt}")
                    if d == 0:
                        nc.scalar.activation(pt, s_ps, EXP, scale=SCALE)
                    else:
                        nc.scalar.activation(pt[:, d:512], s_ps[:, d:512],
                                             EXP, scale=SCALE)
                        nc.scalar.activation(pt[:, 512 + d:1024],
                                             s_ps[:, 512 + d:1024],
                                             EXP, scale=SCALE)
                    nc.tensor.matmul(
                        yA[:, d:512], vt[:, kt, (2 * p) * 65:(2 * p) * 65 + 65],
                        pt[:, d:512],
                        start=(kt == 0), stop=(di == 3),
                        skip_group_check=(di >= 0))
                    nc.tensor.matmul(
                        yB[:, d:512],
                        vt[:, kt, (2 * p + 1) * 65:(2 * p + 1) * 65 + 65],
                        pt[:, 512 + d:1024],
                        start=(kt == 0), stop=(di == 3),
                        skip_group_check=(di >= 0))
                yt = wk.tile([128, 512], BF16, tag="yT", bufs=8,
                             name=f"yT{p}_{c}")
                for h, yps in ((0, yA), (1, yB)):
                    drow = wk.tile([1, 512], F32, tag="rc", bufs=2,
                                   name=f"dr{p}_{c}_{h}")
                    nc.vector.tensor_copy(drow, yps[64:65, :])
                    rc = wk.tile([1, 512], F32, tag="rc", bufs=2,
                                 name=f"rc{p}_{c}_{h}")
                    nc.vector.reciprocal_approx_fast(rc, drow)
                    rr = wk.tile([64, 512], F32, tag="rr", bufs=1,
                                 name=f"rr{p}_{c}_{h}")
                    nc.gpsimd.partition_broadcast(rr, rc)
                    nc.vector.tensor_mul(yt[h * 64:(h + 1) * 64, :],
                                         yps[0:64, :], rr)
                yT[(p, c)] = yt
                ctx.__exit__(None, None, None)

            def cproj_half(c, half):
                # transposed output layout: o^T[oc, t] — bias is then a
                # per-partition scalar (no broadcast tile), yT needs no
                # t4 slicing, and the host transposes the gathered result.
                ctx = nc.named_scope(f"cp{c}_{half}"); ctx.__enter__()
                load_c_consts()
                for ob in range(4 * half, 4 * half + 4):
                    o_ps = ps.tile([128, 512], F32, tag="pj", bufs=2,
                                   name=f"ops{c}_{ob}")
                    for p in range(PAIRS):
                        nc.tensor.matmul(
                            o_ps, wp[:, p, ob, :], yT[(p, c)],
                            start=(p == 0), stop=(p == PAIRS - 1))
                    ot = wk.tile([128, 512], F32, tag="o", bufs=2,
                                 name=f"o{c}_{ob}")
                    nc.vector.tensor_scalar_add(ot, o_ps, bo[:, ob:ob + 1])
                    eng = nc.sync if ob % 2 == 0 else nc.gpsimd
                    eng.dma_start(
                        out_d.ap()[ob * 128:(ob + 1) * 128,
                                   c * CH:(c + 1) * CH], ot)
                if half == 1:
                    for p in range(PAIRS):
                        yT.pop((p, c))
                ctx.__exit__(None, None, None)

            # fine-grained interleave: attention pairs alternate with
            # projection / c_proj slices so the in-order PE queue always has
            # exp-independent matmul work between ACT-dependent ones.
            proj_qk(0, 0)
            proj_qk(0, 1)
            proj_v(0)
            attn_pair(0, 0)
            proj_qk(1, 0)
            attn_pair(0, 1)
            proj_qk(1, 1)
            attn_pair(0, 2)
            proj_v(1)
            attn_pair(0, 3)
            attn_pair(1, 0)
            proj_qk(2, 0)
            attn_pair(1, 1)
            proj_qk(2, 1)
            attn_pair(1, 2)
            proj_v(2)
            attn_pair(1, 3)
            cproj_half(0, 0)
            attn_pair(2, 0)
            cproj_half(0, 1)
            attn_pair(2, 1)
            attn_pair(2, 2)
            proj_qk(3, 0)
            attn_pair(2, 3)
            proj_qk(3, 1)
            proj_v(3)
            cproj_half(1, 0)
            attn_pair(3, 0)
            cproj_half(1, 1)
            attn_pair(3, 1)
            cproj_half(2, 0)
            attn_pair(3, 2)
            cproj_half(2, 1)
            attn_pair(3, 3)
            cproj_half(3, 0)
            cproj_half(3, 1)

    nc.compile()
    return nc


_NC_CACHE = []


def _get_nc():
    if not _NC_CACHE:
        _NC_CACHE.append(build_nc())
    return _NC_CACHE[0]


def _host_consts():
    ident = np.eye(128, dtype=np.float32)
    kk = np.arange(128, dtype=np.int64)[:, None]
    jj = np.arange(128, dtype=np.int64)[None, :]
    tri = np.where(jj < kk, NEG, 0.0).astype(np.float32)
    return ident, tri


def _make_in_maps(x, W_attn, b_attn, W_proj, b_proj):
    ident, tri = _host_consts()
    in_maps = []
    for core in range(8):
        b, hg = core // 2, core % 2
        sl = slice(hg * 512, (hg + 1) * 512)
        w_q = W_attn[:, 0:1024][:, sl]
        w_k = W_attn[:, 1024:2048][:, sl]
        w_v = W_attn[:, 2048:3072][:, sl]
        in_maps.append({
            "x_t": np.ascontiguousarray(x[b].T).astype(BF16NP),
            "w_qk": np.ascontiguousarray(
                np.concatenate([w_q, w_k], axis=1).reshape(8, 128, 8, 128)
                .transpose(2, 1, 0, 3).reshape(1024, 1024)).astype(BF16NP),
            "w_v": np.ascontiguousarray(
                w_v.reshape(8, 128, 512).transpose(1, 0, 2)
                .reshape(128, 4096)).astype(BF16NP),
            "w_p": np.ascontiguousarray(
                W_proj[sl, :].reshape(4, 128, 8, 128).transpose(1, 0, 2, 3)
                .reshape(128, 4096)).astype(BF16NP),
            "b_qk": np.ascontiguousarray(
                np.concatenate([b_attn[0:1024][sl], b_attn[1024:2048][sl]])
                .reshape(8, 128).T),
            "b_v": np.ascontiguousarray(b_attn[2048:3072][sl]),
            "b_o": np.ascontiguousarray(
                (b_proj if hg == 0 else np.zeros_like(b_proj))
                .astype(np.float32).reshape(8, 128).T),
            "ident": ident.astype(BF16NP),
            "masks": tri.astype(BF16NP),
        })
    return in_maps


def _run(inputs, trace=False):
    x = np.asarray(inputs["x"], dtype=np.float32)
    W_attn = np.asarray(inputs["W_attn"], dtype=np.float32)
    b_attn = np.asarray(inputs["b_attn"], dtype=np.float32)
    W_proj = np.asarray(inputs["W_proj"], dtype=np.float32)
    b_proj = np.asarray(inputs["b_proj"], dtype=np.float32)

    nc = _get_nc()
    in_maps = _make_in_maps(x, W_attn, b_attn, W_proj, b_proj)
    res = run_bass_kernel_spmd(nc, in_maps, core_ids=list(range(8)),
                               trace=trace)
    out = np.empty((B, T, C), dtype=np.float32)
    for b in range(B):
        out[b] = (res.results[2 * b]["out_p"]
                  + res.results[2 * b + 1]["out_p"]).T
    return out, res


def kernel(**inputs) -> np.ndarray:
    out, _ = _run(inputs, trace=False)
    return out


# revision 14
# speedup vs baseline: 1.3399x; 1.0177x over previous
"""Causal self-attention kernel for 8 Trainium2 NeuronCores.

Problem: B=4, T=2048, C=1024, NH=16, HD=64 (fp32 in/out).
Sharding: 8 cores = 4 batches x 2 head-groups (8 heads each).
Each core computes qkv projection + causal attention + its partial c_proj
for (batch b, heads hg*8..hg*8+7); host sums the two head-group partials.

v2 (this file) vs the fp32r baseline:
  * x is transposed on the HOST and shipped as bf16 [C,T] — the PE
    transpose phase (128 transposes + 32 PSUM->SBUF copies) is gone.
  * all matmul operands are bf16 (tolerance is 2e-2; measured bf16
    pipeline error ~4e-3).  fp32r already streams 1 cyc/row at N>=256,
    but bf16 keeps 1 cyc/row at the N=128 tiles the trimmed diagonal
    introduces, halves DMA/SBUF, and doubles DVE throughput on 16-bit.
  * causal trimming: for the 4 diagonal k-tiles of each q-chunk the
    fully-masked q-range [0,d) is neither matmul'd, exp'd, nor fed to
    the PV matmul; only the 128-wide true-diagonal block gets a mask
    add (one shared [128,128] triangular mask, via identity matmul).
    Cuts S/exp/PV work in the diagonal region ~45% and the mask adds
    ~70% vs the baseline's full-width mask strips.

On-device dataflow per core (heads processed as 4 pairs of 2):
  q^T,k^T in [feat,T] layout (pair-packed: 2x64 dims = 128 partitions)
  from W^T x^T matmuls; v in [T,feat] layout augmented with a ones
  column per head (softmax denominator rides row 64 of the PV psum).
  S^T[k,q] via two row-packed K=64 matmuls (tile_position); exp on
  ScalarE over the two-head PSUM span; y~^T = v_aug.T @ P^T on PE.
  Normalize with DVE reciprocal + GPSIMD partition_broadcast, then
  c_proj from y^T tiles.  Phase emission interleaves attention pairs
  with projection/c_proj slices so the in-order PE queue always has
  exp-independent matmul work between ACT-dependent ones.
"""

import math

import numpy as np

import concourse.bass as bass
import concourse.mybir as mybir
import concourse.tile as tile
from concourse import bacc
from concourse.bass_utils import run_bass_kernel_spmd

BF16 = mybir.dt.bfloat16
F32 = mybir.dt.float32
EXP = mybir.ActivationFunctionType.Exp
BF16NP = np.dtype(mybir.dt.np(mybir.dt.bfloat16))

B, T, C = 4, 2048, 1024
NH, HD = 16, 64
NHL = 8            # heads per core
PAIRS = 4          # head pairs per core
CH = 512           # q-chunk width
NCH = T // CH      # 4 q-chunks
KT = C // 128      # 8 contraction tiles over C
NTT = T // 128     # 16 T-tiles
SCALE = 1.0 / math.sqrt(HD)
NEG = -1.0e30


def build_nc():
    nc = bacc.Bacc("TRN2", target_bir_lowering=False)

    xt_d = nc.dram_tensor("x_t", [C, T], BF16, kind="ExternalInput")
    wqk_d = nc.dram_tensor("w_qk", [1024, 1024], BF16, kind="ExternalInput")
    wv_d = nc.dram_tensor("w_v", [128, 4096], BF16, kind="ExternalInput")
    wp_d = nc.dram_tensor("w_p", [128, 4096], BF16, kind="ExternalInput")
    bqk_d = nc.dram_tensor("b_qk", [128, 8], F32, kind="ExternalInput")
    bv_d = nc.dram_tensor("b_v", [512], F32, kind="ExternalInput")
    bo_d = nc.dram_tensor("b_o", [128, 8], F32, kind="ExternalInput")
    id_d = nc.dram_tensor("ident", [128, 128], BF16, kind="ExternalInput")
    mask_d = nc.dram_tensor("masks", [128, 128], BF16, kind="ExternalInput")
    out_d = nc.dram_tensor("out_p", [C, T], F32, kind="ExternalOutput")

    with tile.TileContext(nc) as tc:
        with tc.tile_pool(name="cp", bufs=1) as cp, \
             tc.tile_pool(name="wk", bufs=1) as wk, \
             tc.tile_pool(name="ps", bufs=1, space="PSUM") as ps:
            # ---- x^T resident tiles; chunk-0 slices land first, spread
            # across the two HWDGE queues + gpsimd SWDGE ----
            xts = [cp.tile([128, T], BF16, name=f"xt{kt}") for kt in range(KT)]
            for kt in range(KT):
                eng = nc.sync if kt % 2 == 0 else nc.gpsimd
                eng.dma_start(xts[kt][:, 0:CH],
                              xt_d.ap()[kt * 128:(kt + 1) * 128, 0:CH])
            # resident W_qk: one [128,8(kt),128] block per output f-block.
            # f=0 lands in kt-pair pieces so the very first matmuls unblock
            # after ~64KB instead of 256KB.
            wqk_s = cp.tile([128, 8, 8, 128], BF16, name="wqk_s")
            for kt2 in range(4):
                nc.scalar.dma_start(
                    wqk_s[:, 0, 2 * kt2:2 * kt2 + 2, :],
                    wqk_d.ap()[0:128, kt2 * 256:(kt2 + 1) * 256]
                    .rearrange("p (a j) -> p a j", j=128))
            for f in range(1, 8):
                nc.scalar.dma_start(
                    wqk_s[:, f, :, :],
                    wqk_d.ap()[f * 128:(f + 1) * 128, :]
                    .rearrange("p (a j) -> p a j", j=128))
            ident = cp.tile([128, 128], BF16, name="ident")
            nc.sync.dma_start(ident, id_d.ap())
            mask = cp.tile([128, 128], BF16, name="mask")
            bqk = cp.tile([128, 8], F32, name="bqk")
            nc.sync.dma_start(bqk, bqk_d.ap())
            for kt in range(KT):
                eng = nc.sync if kt % 2 == 0 else nc.gpsimd
                eng.dma_start(xts[kt][:, CH:T],
                              xt_d.ap()[kt * 128:(kt + 1) * 128, CH:T])
            wv = cp.tile([128, 8, 512], BF16, name="wv")
            bv_row = cp.tile([1, 512], F32, name="bv_row")
            bv_rep = cp.tile([128, 512], F32, name="bv_rep")
            bo = cp.tile([128, 8], F32, name="bo")
            wp = cp.tile([128, 4, 8, 128], BF16, name="wp")
            consts_loaded = set()

            def load_v_consts():
                if "v" in consts_loaded:
                    return
                consts_loaded.add("v")
                nc.scalar.dma_start(
                    wv, wv_d.ap().rearrange("p (a n) -> p a n", n=512))
                nc.scalar.dma_start(
                    bv_row, bv_d.ap().rearrange("(a n) -> a n", a=1))
                nc.gpsimd.partition_broadcast(bv_rep, bv_row)
                # ones plane for the softmax-denominator columns of v
                nc.gpsimd.memset(
                    vt.rearrange("p t (h e) -> p t h e", e=65)[:, :, :, 64:65],
                    1.0)

            def load_a_consts():
                if "a" in consts_loaded:
                    return
                consts_loaded.add("a")
                nc.scalar.dma_start(mask, mask_d.ap())

            def load_c_consts():
                if "c" in consts_loaded:
                    return
                consts_loaded.add("c")
                nc.scalar.dma_start(
                    wp, wp_d.ap().rearrange("p (a b n) -> p a b n",
                                            a=4, b=8, n=128))
                nc.scalar.dma_start(bo, bo_d.ap())

            # ---- persistent activations ----
            kT = [cp.tile([128, T], BF16, name=f"kT{p}") for p in range(PAIRS)]
            vt = cp.tile([128, NTT, 8 * 65], BF16, name="vt")

            qT = {}   # (pair, chunk) -> [128, 512] tile
            yT = {}   # (pair, chunk) -> [128, 512] tile

            def proj_qk(c, half):
                ctx = nc.named_scope(f"qk{c}_{half}"); ctx.__enter__()
                for f in range(4 * half, 4 * half + 4):
                    qk_ps = ps.tile([128, 512], F32, tag="pj", bufs=2,
                                    name=f"qkps{c}_{f}")
                    for kt in range(KT):
                        nc.tensor.matmul(qk_ps, wqk_s[:, f, kt, :],
                                         xts[kt][:, c * CH:(c + 1) * CH],
                                         start=(kt == 0), stop=(kt == KT - 1))
                    if f < 4:
                        qt = wk.tile([128, 512], BF16, tag="qT", bufs=7,
                                     name=f"qT{f}_{c}")
                        nc.vector.tensor_scalar_add(qt, qk_ps, bqk[:, f:f + 1])
                        qT[(f, c)] = qt
                    else:
                        nc.vector.tensor_scalar_add(
                            kT[f - 4][:, c * CH:(c + 1) * CH], qk_ps,
                            bqk[:, f:f + 1])
                ctx.__exit__(None, None, None)

            def proj_v(c):
                ctx = nc.named_scope(f"v{c}"); ctx.__enter__()
                load_v_consts()
                for t4 in range(4):
                    tt = c * 4 + t4
                    v_ps = ps.tile([128, 512], F32, tag="pj", bufs=2,
                                   name=f"vps{tt}")
                    for kt in range(KT):
                        nc.tensor.matmul(v_ps,
                                         xts[kt][:, tt * 128:(tt + 1) * 128],
                                         wv[:, kt, :],
                                         start=(kt == 0), stop=(kt == KT - 1))
                    vslice = vt[:, tt, :].rearrange("p (h e) -> p h e", e=65)
                    nc.vector.tensor_add(
                        vslice[:, :, 0:64],
                        v_ps.rearrange("p (h e) -> p h e", e=64),
                        bv_rep.rearrange("p (h e) -> p h e", e=64))
                ctx.__exit__(None, None, None)

            def attn_pair(c, p):
                ctx = nc.named_scope(f"at{c}_{p}"); ctx.__enter__()
                load_a_consts()
                nfull = 4 * c
                yA = ps.tile([65, 512], F32, tag="y", bufs=2,
                             name=f"yA{p}_{c}")
                yB = ps.tile([65, 512], F32, tag="y", bufs=2,
                             name=f"yB{p}_{c}")
                qtc = qT.pop((p, c))
                for kt in range(nfull + 4):
                    di = kt - nfull
                    d = 0 if di < 0 else di * 128
                    s_ps = ps.tile([128, 1024], F32, tag="s", bufs=2,
                                   name=f"s{p}_{c}_{kt}")
                    ksl = kT[p][:, kt * 128:(kt + 1) * 128]
                    if di < 0:
                        # off-diagonal: full width, no mask
                        nc.tensor.matmul(s_ps[:, 0:512], ksl[0:64, :],
                                         qtc[0:64, :], start=True, stop=True,
                                         tile_position=(0, 0))
                        nc.tensor.matmul(s_ps[:, 512:1024], ksl[64:128, :],
                                         qtc[64:128, :], start=True, stop=True,
                                         tile_position=(64, 0))
                    else:
                        # diagonal k-tile: q < d is fully masked (skipped);
                        # [d,d+128) is the triangular block (mask add);
                        # [d+128,512) is unmasked.
                        if d + 128 < 512:
                            nc.tensor.matmul(
                                s_ps[:, d + 128:512], ksl[0:64, :],
                                qtc[0:64, d + 128:512], start=True, stop=True,
                                tile_position=(0, 0))
                            nc.tensor.matmul(
                                s_ps[:, 512 + d + 128:1024], ksl[64:128, :],
                                qtc[64:128, d + 128:512], start=True,
                                stop=True, tile_position=(64, 0))
                        nc.tensor.matmul(
                            s_ps[:, d:d + 128], ksl[0:64, :],
                            qtc[0:64, d:d + 128], start=True, stop=False,
                            tile_position=(0, 0))
                        nc.tensor.matmul(
                            s_ps[:, 512 + d:512 + d + 128], ksl[64:128, :],
                            qtc[64:128, d:d + 128], start=True, stop=False,
                            tile_position=(64, 0))
                        nc.tensor.matmul(s_ps[:, d:d + 128], ident, mask,
                                         start=False, stop=True)
                        nc.tensor.matmul(s_ps[:, 512 + d:512 + d + 128],
                                         ident, mask, start=False, stop=True)
                    pt = wk.tile([128, 1024], BF16, tag="P", bufs=3,
                                 name=f"P{p}_{c}_{kt}")
                    if d == 0:
                        nc.scalar.activation(pt, s_ps, EXP, scale=SCALE)
                    else:
                        nc.scalar.activation(pt[:, d:512], s_ps[:, d:512],
                                             EXP, scale=SCALE)
                        nc.scalar.activation(pt[:, 512 + d:1024],
                                             s_ps[:, 512 + d:1024],
                                             EXP, scale=SCALE)
                    nc.tensor.matmul(
                        yA[:, d:512], vt[:, kt, (2 * p) * 65:(2 * p) * 65 + 65],
                        pt[:, d:512],
                        start=(kt == 0), stop=(di == 3),
                        skip_group_check=(di >= 0))
                    nc.tensor.matmul(
                        yB[:, d:512],
                        vt[:, kt, (2 * p + 1) * 65:(2 * p + 1) * 65 + 65],
                        pt[:, 512 + d:1024],
                        start=(kt == 0), stop=(di == 3),
                        skip_group_check=(di >= 0))
                # normalize: recip straight off the PSUM denominator row;
                # A/B chains emitted recip,recip / bcast,bcast / mul,mul so
                # DVE and GPSIMD overlap instead of serializing per head.
                yt = wk.tile([128, 512], BF16, tag="yT", bufs=8,
                             name=f"yT{p}_{c}")
                rcs, rrs = [], []
                for h, yps in ((0, yA), (1, yB)):
                    drow = wk.tile([1, 512], F32, tag="dr", bufs=2,
                                   name=f"dr{p}_{c}_{h}")
                    nc.vector.tensor_copy(drow, yps[64:65, :])
                    rc = wk.tile([1, 512], F32, tag="rc", bufs=2,
                                 name=f"rc{p}_{c}_{h}")
                    nc.vector.reciprocal_approx_fast(rc, drow)
                    rcs.append(rc)
                for h in range(2):
                    rr = wk.tile([64, 512], F32, tag="rr", bufs=2,
                                 name=f"rr{p}_{c}_{h}")
                    nc.gpsimd.partition_broadcast(rr, rcs[h])
                    rrs.append(rr)
                for h, yps in ((0, yA), (1, yB)):
                    nc.vector.tensor_mul(yt[h * 64:(h + 1) * 64, :],
                                         yps[0:64, :], rrs[h])
                yT[(p, c)] = yt
                ctx.__exit__(None, None, None)

            def cproj_half(c, half):
                # transposed output layout: o^T[oc, t] — bias is then a
                # per-partition scalar (no broadcast tile), yT needs no
                # t4 slicing, and the host transposes the gathered result.
                ctx = nc.named_scope(f"cp{c}_{half}"); ctx.__enter__()
                load_c_consts()
                for ob in range(4 * half, 4 * half + 4):
                    o_ps = ps.tile([128, 512], F32, tag="pj", bufs=2,
                                   name=f"ops{c}_{ob}")
                    for p in range(PAIRS):
                        nc.tensor.matmul(
                            o_ps, wp[:, p, ob, :], yT[(p, c)],
                            start=(p == 0), stop=(p == PAIRS - 1))
                    ot = wk.tile([128, 512], F32, tag="o", bufs=2,
                                 name=f"o{c}_{ob}")
                    # bias add on ACT (same table as Exp) — keeps DVE free
                    # for the attention normalize chains.
                    nc.scalar.activation(ot, o_ps,
                                         mybir.ActivationFunctionType.Identity,
                                         bias=bo[:, ob:ob + 1])
                    if c == NCH - 1:
                        # tail: drain the last outputs on three queues
                        engs = (nc.sync, nc.gpsimd, nc.scalar)
                        eng = engs[ob % 3]
                    else:
                        eng = nc.sync if ob % 2 == 0 else nc.gpsimd
                    eng.dma_start(
                        out_d.ap()[ob * 128:(ob + 1) * 128,
                                   c * CH:(c + 1) * CH], ot)
                if half == 1:
                    for p in range(PAIRS):
                        yT.pop((p, c))
                ctx.__exit__(None, None, None)

            # fine-grained interleave: attention pairs alternate with
            # projection / c_proj slices so the in-order PE queue always has
            # exp-independent matmul work between ACT-dependent ones.
            proj_qk(0, 0)
            proj_qk(0, 1)
            proj_v(0)
            attn_pair(0, 0)
            proj_qk(1, 0)
            attn_pair(0, 1)
            proj_qk(1, 1)
            attn_pair(0, 2)
            proj_v(1)
            attn_pair(0, 3)
            attn_pair(1, 0)
            proj_qk(2, 0)
            attn_pair(1, 1)
            proj_qk(2, 1)
            attn_pair(1, 2)
            proj_v(2)
            attn_pair(1, 3)
            cproj_half(0, 0)
            attn_pair(2, 0)
            cproj_half(0, 1)
            attn_pair(2, 1)
            attn_pair(2, 2)
            proj_qk(3, 0)
            attn_pair(2, 3)
            proj_qk(3, 1)
            proj_v(3)
            cproj_half(1, 0)
            attn_pair(3, 0)
            cproj_half(1, 1)
            attn_pair(3, 1)
            cproj_half(2, 0)
            attn_pair(3, 2)
            cproj_half(2, 1)
            attn_pair(3, 3)
            cproj_half(3, 0)
            cproj_half(3, 1)

    nc.compile()
    return nc


_NC_CACHE = []


def _get_nc():
    if not _NC_CACHE:
        _NC_CACHE.append(build_nc())
    return _NC_CACHE[0]


def _host_consts():
    ident = np.eye(128, dtype=np.float32)
    kk = np.arange(128, dtype=np.int64)[:, None]
    jj = np.arange(128, dtype=np.int64)[None, :]
    tri = np.where(jj < kk, NEG, 0.0).astype(np.float32)
    return ident, tri


def _make_in_maps(x, W_attn, b_attn, W_proj, b_proj):
    ident, tri = _host_consts()
    in_maps = []
    for core in range(8):
        b, hg = core // 2, core % 2
        sl = slice(hg * 512, (hg + 1) * 512)
        w_q = W_attn[:, 0:1024][:, sl]
        w_k = W_attn[:, 1024:2048][:, sl]
        w_v = W_attn[:, 2048:3072][:, sl]
        in_maps.append({
            "x_t": np.ascontiguousarray(x[b].T).astype(BF16NP),
            "w_qk": np.ascontiguousarray(
                np.concatenate([w_q, w_k], axis=1).reshape(8, 128, 8, 128)
                .transpose(2, 1, 0, 3).reshape(1024, 1024)).astype(BF16NP),
            "w_v": np.ascontiguousarray(
                w_v.reshape(8, 128, 512).transpose(1, 0, 2)
                .reshape(128, 4096)).astype(BF16NP),
            "w_p": np.ascontiguousarray(
                W_proj[sl, :].reshape(4, 128, 8, 128).transpose(1, 0, 2, 3)
                .reshape(128, 4096)).astype(BF16NP),
            "b_qk": np.ascontiguousarray(
                np.concatenate([b_attn[0:1024][sl], b_attn[1024:2048][sl]])
                .reshape(8, 128).T),
            "b_v": np.ascontiguousarray(b_attn[2048:3072][sl]),
            "b_o": np.ascontiguousarray(
                (b_proj if hg == 0 else np.zeros_like(b_proj))
                .astype(np.float32).reshape(8, 128).T),
            "ident": ident.astype(BF16NP),
            "masks": tri.astype(BF16NP),
        })
    return in_maps


def _run(inputs, trace=False):
    x = np.asarray(inputs["x"], dtype=np.float32)
    W_attn = np.asarray(inputs["W_attn"], dtype=np.float32)
    b_attn = np.asarray(inputs["b_attn"], dtype=np.float32)
    W_proj = np.asarray(inputs["W_proj"], dtype=np.float32)
    b_proj = np.asarray(inputs["b_proj"], dtype=np.float32)

    nc = _get_nc()
    in_maps = _make_in_maps(x, W_attn, b_attn, W_proj, b_proj)
    res = run_bass_kernel_spmd(nc, in_maps, core_ids=list(range(8)),
                               trace=trace)
    out = np.empty((B, T, C), dtype=np.float32)
    for b in range(B):
        out[b] = (res.results[2 * b]["out_p"]
                  + res.results[2 * b + 1]["out_p"]).T
    return out, res


def kernel(**inputs) -> np.ndarray:
    out, _ = _run(inputs, trace=False)
    return out


# revision 20
# speedup vs baseline: 1.3591x; 1.0143x over previous
"""Causal self-attention kernel for 8 Trainium2 NeuronCores.

Problem: B=4, T=2048, C=1024, NH=16, HD=64 (fp32 in/out).
Sharding: 8 cores = 4 batches x 2 head-groups (8 heads each).
Each core computes qkv projection + causal attention + its partial c_proj
for (batch b, heads hg*8..hg*8+7); host sums the two head-group partials.

v2 (this file) vs the fp32r baseline:
  * x is transposed on the HOST and shipped as bf16 [C,T] — the PE
    transpose phase (128 transposes + 32 PSUM->SBUF copies) is gone.
  * all matmul operands are bf16 (tolerance is 2e-2; measured bf16
    pipeline error ~4e-3).  fp32r already streams 1 cyc/row at N>=256,
    but bf16 keeps 1 cyc/row at the N=128 tiles the trimmed diagonal
    introduces, halves DMA/SBUF, and doubles DVE throughput on 16-bit.
  * causal trimming: for the 4 diagonal k-tiles of each q-chunk the
    fully-masked q-range [0,d) is neither matmul'd, exp'd, nor fed to
    the PV matmul; only the 128-wide true-diagonal block gets a mask
    add (one shared [128,128] triangular mask, via identity matmul).
    Cuts S/exp/PV work in the diagonal region ~45% and the mask adds
    ~70% vs the baseline's full-width mask strips.

On-device dataflow per core (heads processed as 4 pairs of 2):
  q^T,k^T in [feat,T] layout (pair-packed: 2x64 dims = 128 partitions)
  from W^T x^T matmuls; v in [T,feat] layout augmented with a ones
  column per head (softmax denominator rides row 64 of the PV psum).
  S^T[k,q] via two row-packed K=64 matmuls (tile_position); exp on
  ScalarE over the two-head PSUM span; y~^T = v_aug.T @ P^T on PE.
  Normalize with DVE reciprocal + GPSIMD partition_broadcast, then
  c_proj from y^T tiles.  Phase emission interleaves attention pairs
  with projection/c_proj slices so the in-order PE queue always has
  exp-independent matmul work between ACT-dependent ones.
"""

import math

import numpy as np

import concourse.bass as bass
import concourse.mybir as mybir
import concourse.tile as tile
from concourse import bacc
from concourse.bass_utils import run_bass_kernel_spmd

BF16 = mybir.dt.bfloat16
F32 = mybir.dt.float32
EXP = mybir.ActivationFunctionType.Exp
BF16NP = np.dtype(mybir.dt.np(mybir.dt.bfloat16))

B, T, C = 4, 2048, 1024
NH, HD = 16, 64
NHL = 8            # heads per core
PAIRS = 4          # head pairs per core
CH = 512           # q-chunk width
NCH = T // CH      # 4 q-chunks
KT = C // 128      # 8 contraction tiles over C
NTT = T // 128     # 16 T-tiles
SCALE = 1.0 / math.sqrt(HD)
NEG = -1.0e30


def build_nc():
    nc = bacc.Bacc("TRN2", target_bir_lowering=False)

    xt_d = nc.dram_tensor("x_t", [C, T], BF16, kind="ExternalInput")
    wqk_d = nc.dram_tensor("w_qk", [1024, 1024], BF16, kind="ExternalInput")
    wv_d = nc.dram_tensor("w_v", [128, 4096], BF16, kind="ExternalInput")
    wp_d = nc.dram_tensor("w_p", [128, 4096], BF16, kind="ExternalInput")
    bqk_d = nc.dram_tensor("b_qk", [128, 8], F32, kind="ExternalInput")
    bv_d = nc.dram_tensor("b_v", [512], F32, kind="ExternalInput")
    bo_d = nc.dram_tensor("b_o", [128, 8], F32, kind="ExternalInput")
    id_d = nc.dram_tensor("ident", [128, 128], BF16, kind="ExternalInput")
    mask_d = nc.dram_tensor("masks", [128, 128], BF16, kind="ExternalInput")
    out_d = nc.dram_tensor("out_p", [C, T], F32, kind="ExternalOutput")

    with tile.TileContext(nc) as tc:
        with tc.tile_pool(name="cp", bufs=1) as cp, \
             tc.tile_pool(name="wk", bufs=1) as wk, \
             tc.tile_pool(name="ps", bufs=1, space="PSUM") as ps:
            # ---- x^T resident tiles; chunk-0 slices land first, spread
            # across the two HWDGE queues + gpsimd SWDGE ----
            xts = [cp.tile([128, T], BF16, name=f"xt{kt}") for kt in range(KT)]
            for kt in range(KT):
                eng = nc.sync if kt % 2 == 0 else nc.gpsimd
                eng.dma_start(xts[kt][:, 0:CH],
                              xt_d.ap()[kt * 128:(kt + 1) * 128, 0:CH])
            # resident W_qk: one [128,8(kt),128] block per output f-block.
            # f=0 lands in kt-pair pieces so the very first matmuls unblock
            # after ~64KB instead of 256KB.
            wqk_s = cp.tile([128, 8, 8, 128], BF16, name="wqk_s")
            for kt2 in range(4):
                nc.scalar.dma_start(
                    wqk_s[:, 0, 2 * kt2:2 * kt2 + 2, :],
                    wqk_d.ap()[0:128, kt2 * 256:(kt2 + 1) * 256]
                    .rearrange("p (a j) -> p a j", j=128))
            for f in range(1, 8):
                nc.scalar.dma_start(
                    wqk_s[:, f, :, :],
                    wqk_d.ap()[f * 128:(f + 1) * 128, :]
                    .rearrange("p (a j) -> p a j", j=128))
            ident = cp.tile([128, 128], BF16, name="ident")
            nc.sync.dma_start(ident, id_d.ap())
            mask = cp.tile([128, 128], BF16, name="mask")
            nc.sync.dma_start(mask, mask_d.ap())
            bqk = cp.tile([128, 8], F32, name="bqk")
            nc.sync.dma_start(bqk, bqk_d.ap())
            for kt in range(KT):
                eng = nc.sync if kt % 2 == 0 else nc.gpsimd
                eng.dma_start(xts[kt][:, CH:T],
                              xt_d.ap()[kt * 128:(kt + 1) * 128, CH:T])
            # v-path constants up front: wv must beat proj_v's first matmul,
            # and DMA issue sits in the ACT queue behind any emitted exps.
            wv = cp.tile([128, 8, 512], BF16, name="wv")
            nc.scalar.dma_start(
                wv, wv_d.ap().rearrange("p (a n) -> p a n", n=512))
            bv_row = cp.tile([1, 512], F32, name="bv_row")
            nc.scalar.dma_start(
                bv_row, bv_d.ap().rearrange("(a n) -> a n", a=1))
            bv_rep = cp.tile([128, 512], F32, name="bv_rep")
            nc.gpsimd.partition_broadcast(bv_rep, bv_row)
            bo = cp.tile([128, 8], F32, name="bo")
            wp = cp.tile([128, 4, 8, 128], BF16, name="wp")
            consts_loaded = set()

            def load_v_consts():
                if "v" in consts_loaded:
                    return
                consts_loaded.add("v")
                # ones plane for the softmax-denominator columns of v
                nc.gpsimd.memset(
                    vt.rearrange("p t (h e) -> p t h e", e=65)[:, :, :, 64:65],
                    1.0)

            def load_c_consts():
                if "c" in consts_loaded:
                    return
                consts_loaded.add("c")
                nc.scalar.dma_start(
                    wp, wp_d.ap().rearrange("p (a b n) -> p a b n",
                                            a=4, b=8, n=128))
                nc.scalar.dma_start(bo, bo_d.ap())

            # ---- persistent activations ----
            kT = [cp.tile([128, T], BF16, name=f"kT{p}") for p in range(PAIRS)]
            vt = cp.tile([128, NTT, 8 * 65], BF16, name="vt")

            qT = {}   # (pair, chunk) -> [128, 512] tile
            yT = {}   # (pair, chunk) -> [128, 512] tile

            def proj_qk(c, half):
                # generator: one unit = 2 matmuls (~0.85us PE)
                for f in range(4 * half, 4 * half + 4):
                    qk_ps = ps.tile([128, 512], F32, tag="pj", bufs=2,
                                    name=f"qkps{c}_{f}")
                    for kt2 in range(KT // 2):
                        ctx = nc.named_scope(f"qk{c}_{half}"); ctx.__enter__()
                        if (c, half) == (2, 0):
                            # prefetch c_proj consts well before cp(0,0)
                            load_c_consts()
                        for kt in (2 * kt2, 2 * kt2 + 1):
                            nc.tensor.matmul(qk_ps, wqk_s[:, f, kt, :],
                                             xts[kt][:, c * CH:(c + 1) * CH],
                                             start=(kt == 0),
                                             stop=(kt == KT - 1))
                        if kt2 == KT // 2 - 1:
                            if f < 4:
                                qt = wk.tile([128, 512], BF16, tag="qT",
                                             bufs=12, name=f"qT{f}_{c}")
                                nc.vector.tensor_scalar_add(qt, qk_ps,
                                                            bqk[:, f:f + 1])
                                qT[(f, c)] = qt
                            else:
                                nc.vector.tensor_scalar_add(
                                    kT[f - 4][:, c * CH:(c + 1) * CH], qk_ps,
                                    bqk[:, f:f + 1])
                        ctx.__exit__(None, None, None)
                        yield

            def proj_v(c):
                first = True
                for t4 in range(4):
                    tt = c * 4 + t4
                    v_ps = ps.tile([128, 512], F32, tag="pj", bufs=2,
                                   name=f"vps{tt}")
                    for kt2 in range(KT // 2):
                        ctx = nc.named_scope(f"v{c}"); ctx.__enter__()
                        if first:
                            load_v_consts()
                            first = False
                        for kt in (2 * kt2, 2 * kt2 + 1):
                            nc.tensor.matmul(
                                v_ps, xts[kt][:, tt * 128:(tt + 1) * 128],
                                wv[:, kt, :],
                                start=(kt == 0), stop=(kt == KT - 1))
                        if kt2 == KT // 2 - 1:
                            vslice = vt[:, tt, :].rearrange(
                                "p (h e) -> p h e", e=65)
                            nc.vector.tensor_add(
                                vslice[:, :, 0:64],
                                v_ps.rearrange("p (h e) -> p h e", e=64),
                                bv_rep.rearrange("p (h e) -> p h e", e=64))
                        ctx.__exit__(None, None, None)
                        yield

            def attn_pair(c, p):
                # generator: one unit = one k-tile (S + exp + PV), plus a
                # final normalize unit
                nfull = 4 * c
                ctx = nc.named_scope(f"at{c}_{p}"); ctx.__enter__()
                yA = ps.tile([65, 512], F32, tag="y", bufs=2,
                             name=f"yA{p}_{c}")
                yB = ps.tile([65, 512], F32, tag="y", bufs=2,
                             name=f"yB{p}_{c}")
                qtc = qT.pop((p, c))
                ctx.__exit__(None, None, None)
                for kt in range(nfull + 4):
                    ctx = nc.named_scope(f"at{c}_{p}"); ctx.__enter__()
                    di = kt - nfull
                    d = 0 if di < 0 else di * 128
                    s_ps = ps.tile([128, 1024], F32, tag="s", bufs=2,
                                   name=f"s{p}_{c}_{kt}")
                    ksl = kT[p][:, kt * 128:(kt + 1) * 128]
                    if di < 0:
                        # off-diagonal: full width, no mask
                        nc.tensor.matmul(s_ps[:, 0:512], ksl[0:64, :],
                                         qtc[0:64, :], start=True, stop=True,
                                         tile_position=(0, 0))
                        nc.tensor.matmul(s_ps[:, 512:1024], ksl[64:128, :],
                                         qtc[64:128, :], start=True, stop=True,
                                         tile_position=(64, 0))
                    else:
                        # diagonal k-tile: q < d is fully masked (skipped);
                        # [d,d+128) is the triangular block (mask add);
                        # [d+128,512) is unmasked.
                        if d + 128 < 512:
                            nc.tensor.matmul(
                                s_ps[:, d + 128:512], ksl[0:64, :],
                                qtc[0:64, d + 128:512], start=True, stop=True,
                                tile_position=(0, 0))
                            nc.tensor.matmul(
                                s_ps[:, 512 + d + 128:1024], ksl[64:128, :],
                                qtc[64:128, d + 128:512], start=True,
                                stop=True, tile_position=(64, 0))
                        nc.tensor.matmul(
                            s_ps[:, d:d + 128], ksl[0:64, :],
                            qtc[0:64, d:d + 128], start=True, stop=False,
                            tile_position=(0, 0))
                        nc.tensor.matmul(
                            s_ps[:, 512 + d:512 + d + 128], ksl[64:128, :],
                            qtc[64:128, d:d + 128], start=True, stop=False,
                            tile_position=(64, 0))
                        nc.tensor.matmul(s_ps[:, d:d + 128], ident, mask,
                                         start=False, stop=True)
                        nc.tensor.matmul(s_ps[:, 512 + d:512 + d + 128],
                                         ident, mask, start=False, stop=True)
                    pt = wk.tile([128, 1024], BF16, tag="P", bufs=3,
                                 name=f"P{p}_{c}_{kt}")
                    if d == 0:
                        nc.scalar.activation(pt, s_ps, EXP, scale=SCALE)
                    else:
                        nc.scalar.activation(pt[:, d:512], s_ps[:, d:512],
                                             EXP, scale=SCALE)
                        nc.scalar.activation(pt[:, 512 + d:1024],
                                             s_ps[:, 512 + d:1024],
                                             EXP, scale=SCALE)
                    nc.tensor.matmul(
                        yA[:, d:512], vt[:, kt, (2 * p) * 65:(2 * p) * 65 + 65],
                        pt[:, d:512],
                        start=(kt == 0), stop=(di == 3),
                        skip_group_check=(di >= 0))
                    nc.tensor.matmul(
                        yB[:, d:512],
                        vt[:, kt, (2 * p + 1) * 65:(2 * p + 1) * 65 + 65],
                        pt[:, 512 + d:1024],
                        start=(kt == 0), stop=(di == 3),
                        skip_group_check=(di >= 0))
                    ctx.__exit__(None, None, None)
                    yield
                # normalize unit: A/B chains emitted recip,recip /
                # bcast,bcast / mul,mul so DVE and GPSIMD overlap.
                ctx = nc.named_scope(f"at{c}_{p}"); ctx.__enter__()
                yt = wk.tile([128, 512], BF16, tag="yT", bufs=10,
                             name=f"yT{p}_{c}")
                rcs, rrs = [], []
                for h, yps in ((0, yA), (1, yB)):
                    drow = wk.tile([1, 512], F32, tag="dr", bufs=2,
                                   name=f"dr{p}_{c}_{h}")
                    nc.vector.tensor_copy(drow, yps[64:65, :])
                    rc = wk.tile([1, 512], F32, tag="rc", bufs=2,
                                 name=f"rc{p}_{c}_{h}")
                    nc.vector.reciprocal_approx_fast(rc, drow)
                    rcs.append(rc)
                for h in range(2):
                    rr = wk.tile([64, 512], F32, tag="rr", bufs=2,
                                 name=f"rr{p}_{c}_{h}")
                    nc.gpsimd.partition_broadcast(rr, rcs[h])
                    rrs.append(rr)
                for h, yps in ((0, yA), (1, yB)):
                    nc.vector.tensor_mul(yt[h * 64:(h + 1) * 64, :],
                                         yps[0:64, :], rrs[h])
                yT[(p, c)] = yt
                ctx.__exit__(None, None, None)
                yield

            def cproj_half(c, half):
                # transposed output layout: o^T[oc, t] — bias is a
                # per-partition scalar handled on ACT (same table as Exp),
                # yT needs no t4 slicing; the host transposes the result.
                # One unit = 2 matmuls.
                for ob in range(4 * half, 4 * half + 4):
                    o_ps = ps.tile([128, 512], F32, tag="pj", bufs=2,
                                   name=f"ops{c}_{ob}")
                    for pg in range(2):
                        ctx = nc.named_scope(f"cp{c}_{half}"); ctx.__enter__()
                        load_c_consts()
                        for p in (2 * pg, 2 * pg + 1):
                            nc.tensor.matmul(
                                o_ps, wp[:, p, ob, :], yT[(p, c)],
                                start=(p == 0), stop=(p == PAIRS - 1))
                        if pg == 1:
                            ot = wk.tile([128, 512], F32, tag="o", bufs=2,
                                         name=f"o{c}_{ob}")
                            nc.scalar.activation(
                                ot, o_ps,
                                mybir.ActivationFunctionType.Identity,
                                bias=bo[:, ob:ob + 1])
                            if c == NCH - 1:
                                engs = (nc.sync, nc.gpsimd, nc.scalar)
                                eng = engs[ob % 3]
                            else:
                                eng = nc.sync if ob % 2 == 0 else nc.gpsimd
                            eng.dma_start(
                                out_d.ap()[ob * 128:(ob + 1) * 128,
                                           c * CH:(c + 1) * CH], ot)
                        ctx.__exit__(None, None, None)
                        yield
                if half == 1:
                    for p in range(PAIRS):
                        yT.pop((p, c))

            # ---- unit-level scheduler ----
            # Stream A: attention pairs in order.  Stream B: projections +
            # c_proj.  Units are interleaved so the in-order PE queue always
            # holds exp-independent matmul work between ACT-dependent ones
            # (keeps the PE pstate ramped and hides the S->exp->PV latency).
            # Gates: attn(c,*) may not start before qk(c,*) and v(c) are
            # fully emitted (PE in-order would deadlock otherwise);
            # cproj(c,*) may not start before attn(c,3) is done.
            a_phases = [(c, p) for c in range(NCH) for p in range(PAIRS)]
            b_phases = ([("qk", 0, 0), ("qk", 0, 1), ("v", 0)] +
                        [("qk", 1, 0), ("qk", 1, 1), ("v", 1),
                         ("qk", 2, 0), ("qk", 2, 1), ("v", 2),
                         ("cp", 0, 0), ("cp", 0, 1),
                         ("qk", 3, 0), ("qk", 3, 1), ("v", 3),
                         ("cp", 1, 0), ("cp", 1, 1),
                         ("cp", 2, 0), ("cp", 2, 1),
                         ("cp", 3, 0), ("cp", 3, 1)])

            def b_units(ph):
                return {"qk": 16, "v": 16, "cp": 8}[ph[0]]

            def make_b(ph):
                kind = ph[0]
                if kind == "qk":
                    return proj_qk(ph[1], ph[2])
                if kind == "v":
                    return proj_v(ph[1])
                return cproj_half(ph[1], ph[2])

            b_done = set()    # finished b phases
            a_done = set()    # finished attn pairs
            rem_a = sum(4 * c + 5 for c, p in a_phases)
            rem_b = sum(b_units(ph) for ph in b_phases)
            ai, bi = 0, 0
            a_gen = b_gen = None
            bal = 0.0
            while True:
                can_a = a_gen is not None or (
                    ai < len(a_phases)
                    and ("v", a_phases[ai][0]) in b_done)
                can_b = b_gen is not None or (
                    bi < len(b_phases)
                    and (b_phases[bi][0] != "cp"
                         or (b_phases[bi][1], 3) in a_done))
                if not can_a and not can_b:
                    if ai >= len(a_phases) and bi >= len(b_phases):
                        break
                    raise RuntimeError("scheduler deadlock")
                take_b = can_b and (not can_a or bal >= 1.0)
                if take_b:
                    if b_gen is None:
                        b_gen = make_b(b_phases[bi])
                    try:
                        next(b_gen)
                        rem_b -= 1
                        if bal >= 1.0:
                            bal -= 1.0
                    except StopIteration:
                        b_done.add(b_phases[bi][:2] if b_phases[bi][0] != "qk"
                                   else b_phases[bi])
                        b_done.add(b_phases[bi])
                        bi += 1
                        b_gen = None
                else:
                    if a_gen is None:
                        a_gen = attn_pair(*a_phases[ai])
                    try:
                        next(a_gen)
                        rem_a -= 1
                        bal += rem_b / max(rem_a, 1)
                    except StopIteration:
                        a_done.add(a_phases[ai])
                        ai += 1
                        a_gen = None

    nc.compile()
    return nc


_NC_CACHE = []


def _get_nc():
    if not _NC_CACHE:
        _NC_CACHE.append(build_nc())
    return _NC_CACHE[0]


def _host_consts():
    ident = np.eye(128, dtype=np.float32)
    kk = np.arange(128, dtype=np.int64)[:, None]
    jj = np.arange(128, dtype=np.int64)[None, :]
    tri = np.where(jj < kk, NEG, 0.0).astype(np.float32)
    return ident, tri


def _make_in_maps(x, W_attn, b_attn, W_proj, b_proj):
    ident, tri = _host_consts()
    in_maps = []
    for core in range(8):
        b, hg = core // 2, core % 2
        sl = slice(hg * 512, (hg + 1) * 512)
        w_q = W_attn[:, 0:1024][:, sl]
        w_k = W_attn[:, 1024:2048][:, sl]
        w_v = W_attn[:, 2048:3072][:, sl]
        in_maps.append({
            "x_t": np.ascontiguousarray(x[b].T).astype(BF16NP),
            "w_qk": np.ascontiguousarray(
                np.concatenate([w_q, w_k], axis=1).reshape(8, 128, 8, 128)
                .transpose(2, 1, 0, 3).reshape(1024, 1024)).astype(BF16NP),
            "w_v": np.ascontiguousarray(
                w_v.reshape(8, 128, 512).transpose(1, 0, 2)
                .reshape(128, 4096)).astype(BF16NP),
            "w_p": np.ascontiguousarray(
                W_proj[sl, :].reshape(4, 128, 8, 128).transpose(1, 0, 2, 3)
                .reshape(128, 4096)).astype(BF16NP),
            "b_qk": np.ascontiguousarray(
                np.concatenate([b_attn[0:1024][sl], b_attn[1024:2048][sl]])
                .reshape(8, 128).T),
            "b_v": np.ascontiguousarray(b_attn[2048:3072][sl]),
            "b_o": np.ascontiguousarray(
                (b_proj if hg == 0 else np.zeros_like(b_proj))
                .astype(np.float32).reshape(8, 128).T),
            "ident": ident.astype(BF16NP),
            "masks": tri.astype(BF16NP),
        })
    return in_maps


def _run(inputs, trace=False):
    x = np.asarray(inputs["x"], dtype=np.float32)
    W_attn = np.asarray(inputs["W_attn"], dtype=np.float32)
    b_attn = np.asarray(inputs["b_attn"], dtype=np.float32)
    W_proj = np.asarray(inputs["W_proj"], dtype=np.float32)
    b_proj = np.asarray(inputs["b_proj"], dtype=np.float32)

    nc = _get_nc()
    in_maps = _make_in_maps(x, W_attn, b_attn, W_proj, b_proj)
    res = run_bass_kernel_spmd(nc, in_maps, core_ids=list(range(8)),
                               trace=trace)
    out = np.empty((B, T, C), dtype=np.float32)
    for b in range(B):
        out[b] = (res.results[2 * b]["out_p"]
                  + res.results[2 * b + 1]["out_p"]).T
    return out, res


def kernel(**inputs) -> np.ndarray:
    out, _ = _run(inputs, trace=False)
    return out


# revision 25
# speedup vs baseline: 1.3633x; 1.0031x over previous
"""Causal self-attention kernel for 8 Trainium2 NeuronCores.

Problem: B=4, T=2048, C=1024, NH=16, HD=64 (fp32 in/out).
Sharding: 8 cores = 4 batches x 2 head-groups (8 heads each).
Each core computes qkv projection + causal attention + its partial c_proj
for (batch b, heads hg*8..hg*8+7); host sums the two head-group partials.

v2 (this file) vs the fp32r baseline:
  * x is transposed on the HOST and shipped as bf16 [C,T] — the PE
    transpose phase (128 transposes + 32 PSUM->SBUF copies) is gone.
  * all matmul operands are bf16 (tolerance is 2e-2; measured bf16
    pipeline error ~4e-3).  fp32r already streams 1 cyc/row at N>=256,
    but bf16 keeps 1 cyc/row at the N=128 tiles the trimmed diagonal
    introduces, halves DMA/SBUF, and doubles DVE throughput on 16-bit.
  * causal trimming: for the 4 diagonal k-tiles of each q-chunk the
    fully-masked q-range [0,d) is neither matmul'd, exp'd, nor fed to
    the PV matmul; only the 128-wide true-diagonal block gets a mask
    add (one shared [128,128] triangular mask, via identity matmul).
    Cuts S/exp/PV work in the diagonal region ~45% and the mask adds
    ~70% vs the baseline's full-width mask strips.

On-device dataflow per core (heads processed as 4 pairs of 2):
  q^T,k^T in [feat,T] layout (pair-packed: 2x64 dims = 128 partitions)
  from W^T x^T matmuls; v in [T,feat] layout augmented with a ones
  column per head (softmax denominator rides row 64 of the PV psum).
  S^T[k,q] via two row-packed K=64 matmuls (tile_position); exp on
  ScalarE over the two-head PSUM span; y~^T = v_aug.T @ P^T on PE.
  Normalize with DVE reciprocal + GPSIMD partition_broadcast, then
  c_proj from y^T tiles.  Phase emission interleaves attention pairs
  with projection/c_proj slices so the in-order PE queue always has
  exp-independent matmul work between ACT-dependent ones.
"""

import math

import numpy as np

import concourse.bass as bass
import concourse.mybir as mybir
import concourse.tile as tile
from concourse import bacc
from concourse.bass_utils import run_bass_kernel_spmd

BF16 = mybir.dt.bfloat16
F32 = mybir.dt.float32
EXP = mybir.ActivationFunctionType.Exp
BF16NP = np.dtype(mybir.dt.np(mybir.dt.bfloat16))

B, T, C = 4, 2048, 1024
NH, HD = 16, 64
NHL = 8            # heads per core
PAIRS = 4          # head pairs per core
CH = 512           # q-chunk width
NCH = T // CH      # 4 q-chunks
KT = C // 128      # 8 contraction tiles over C
NTT = T // 128     # 16 T-tiles
SCALE = 1.0 / math.sqrt(HD)
NEG = -1.0e30


def build_nc():
    nc = bacc.Bacc("TRN2", target_bir_lowering=False)

    xt_d = nc.dram_tensor("x_t", [C, T], BF16, kind="ExternalInput")
    wqk_d = nc.dram_tensor("w_qk", [1024, 1024], BF16, kind="ExternalInput")
    wv_d = nc.dram_tensor("w_v", [128, 4096], BF16, kind="ExternalInput")
    wp_d = nc.dram_tensor("w_p", [128, 4096], BF16, kind="ExternalInput")
    bqk_d = nc.dram_tensor("b_qk", [128, 8], F32, kind="ExternalInput")
    bv_d = nc.dram_tensor("b_v", [512], F32, kind="ExternalInput")
    bo_d = nc.dram_tensor("b_o", [128, 8], F32, kind="ExternalInput")
    id_d = nc.dram_tensor("ident", [128, 128], BF16, kind="ExternalInput")
    mask_d = nc.dram_tensor("masks", [128, 128], BF16, kind="ExternalInput")
    out_d = nc.dram_tensor("out_p", [C, T], F32, kind="ExternalOutput")

    with tile.TileContext(nc) as tc:
        with tc.tile_pool(name="cp", bufs=1) as cp, \
             tc.tile_pool(name="wk", bufs=1) as wk, \
             tc.tile_pool(name="ps", bufs=1, space="PSUM") as ps:
            # ---- x^T resident tiles; chunk-0 slices land first, spread
            # across the two HWDGE queues + gpsimd SWDGE ----
            xts = [cp.tile([128, T], BF16, name=f"xt{kt}") for kt in range(KT)]
            for kt in range(KT):
                eng = nc.sync if kt % 2 == 0 else nc.gpsimd
                eng.dma_start(xts[kt][:, 0:CH],
                              xt_d.ap()[kt * 128:(kt + 1) * 128, 0:CH])
            # resident W_qk: one [128,8(kt),128] block per output f-block.
            # f=0 lands in kt-pair pieces so the very first matmuls unblock
            # after ~64KB instead of 256KB.
            wqk_s = cp.tile([128, 8, 8, 128], BF16, name="wqk_s")
            for kt2 in range(4):
                nc.scalar.dma_start(
                    wqk_s[:, 0, 2 * kt2:2 * kt2 + 2, :],
                    wqk_d.ap()[0:128, kt2 * 256:(kt2 + 1) * 256]
                    .rearrange("p (a j) -> p a j", j=128))
            for f in range(1, 8):
                nc.scalar.dma_start(
                    wqk_s[:, f, :, :],
                    wqk_d.ap()[f * 128:(f + 1) * 128, :]
                    .rearrange("p (a j) -> p a j", j=128))
            ident = cp.tile([128, 128], BF16, name="ident")
            nc.sync.dma_start(ident, id_d.ap())
            mask = cp.tile([128, 128], BF16, name="mask")
            nc.sync.dma_start(mask, mask_d.ap())
            bqk = cp.tile([128, 8], F32, name="bqk")
            nc.sync.dma_start(bqk, bqk_d.ap())
            for kt in range(KT):
                eng = nc.sync if kt % 2 == 0 else nc.gpsimd
                eng.dma_start(xts[kt][:, CH:T],
                              xt_d.ap()[kt * 128:(kt + 1) * 128, CH:T])
            # v-path constants up front: wv must beat proj_v's first matmul,
            # and DMA issue sits in the ACT queue behind any emitted exps.
            wv = cp.tile([128, 8, 512], BF16, name="wv")
            nc.scalar.dma_start(
                wv, wv_d.ap().rearrange("p (a n) -> p a n", n=512))
            bv_row = cp.tile([1, 512], F32, name="bv_row")
            nc.scalar.dma_start(
                bv_row, bv_d.ap().rearrange("(a n) -> a n", a=1))
            bv_rep = cp.tile([128, 512], F32, name="bv_rep")
            nc.gpsimd.partition_broadcast(bv_rep, bv_row)
            bo = cp.tile([128, 8], F32, name="bo")
            wp = cp.tile([128, 4, 8, 128], BF16, name="wp")
            consts_loaded = set()

            def load_v_consts():
                if "v" in consts_loaded:
                    return
                consts_loaded.add("v")
                # ones plane for the softmax-denominator columns of v
                nc.gpsimd.memset(
                    vt.rearrange("p t (h e) -> p t h e", e=65)[:, :, :, 64:65],
                    1.0)

            def load_c_consts():
                if "c" in consts_loaded:
                    return
                consts_loaded.add("c")
                nc.scalar.dma_start(
                    wp, wp_d.ap().rearrange("p (a b n) -> p a b n",
                                            a=4, b=8, n=128))
                nc.scalar.dma_start(bo, bo_d.ap())

            # ---- persistent activations ----
            kT = [cp.tile([128, T], BF16, name=f"kT{p}") for p in range(PAIRS)]
            vt = cp.tile([128, NTT, 8 * 65], BF16, name="vt")

            qT = {}   # (pair, chunk) -> [128, 512] tile
            yT = {}   # (pair, chunk) -> [128, 512] tile

            def proj_qk(c, half):
                # generator: one unit = 2 matmuls (~0.85us PE)
                for f in range(4 * half, 4 * half + 4):
                    qk_ps = ps.tile([128, 512], F32, tag="pj", bufs=2,
                                    name=f"qkps{c}_{f}")
                    for kt2 in range(KT // 2):
                        ctx = nc.named_scope(f"qk{c}_{half}"); ctx.__enter__()
                        if (c, half) == (2, 0):
                            # prefetch c_proj consts well before cp(0,0)
                            load_c_consts()
                        for kt in (2 * kt2, 2 * kt2 + 1):
                            nc.tensor.matmul(qk_ps, wqk_s[:, f, kt, :],
                                             xts[kt][:, c * CH:(c + 1) * CH],
                                             start=(kt == 0),
                                             stop=(kt == KT - 1))
                        if kt2 == KT // 2 - 1:
                            if f < 4:
                                qt = wk.tile([128, 512], BF16, tag="qT",
                                             bufs=12, name=f"qT{f}_{c}")
                                nc.vector.tensor_scalar_add(qt, qk_ps,
                                                            bqk[:, f:f + 1])
                                qT[(f, c)] = qt
                            else:
                                nc.vector.tensor_scalar_add(
                                    kT[f - 4][:, c * CH:(c + 1) * CH], qk_ps,
                                    bqk[:, f:f + 1])
                        ctx.__exit__(None, None, None)
                        yield

            def proj_v(c):
                first = True
                for t4 in range(4):
                    tt = c * 4 + t4
                    v_ps = ps.tile([128, 512], F32, tag="pj", bufs=2,
                                   name=f"vps{tt}")
                    for kt2 in range(KT // 2):
                        ctx = nc.named_scope(f"v{c}"); ctx.__enter__()
                        if first:
                            load_v_consts()
                            first = False
                        for kt in (2 * kt2, 2 * kt2 + 1):
                            nc.tensor.matmul(
                                v_ps, xts[kt][:, tt * 128:(tt + 1) * 128],
                                wv[:, kt, :],
                                start=(kt == 0), stop=(kt == KT - 1))
                        if kt2 == KT // 2 - 1:
                            vslice = vt[:, tt, :].rearrange(
                                "p (h e) -> p h e", e=65)
                            nc.vector.tensor_add(
                                vslice[:, :, 0:64],
                                v_ps.rearrange("p (h e) -> p h e", e=64),
                                bv_rep.rearrange("p (h e) -> p h e", e=64))
                        ctx.__exit__(None, None, None)
                        yield

            def attn_pair(c, p):
                # generator: one unit = one k-tile (S + exp + PV), plus a
                # final normalize unit
                nfull = 4 * c
                ctx = nc.named_scope(f"at{c}_{p}"); ctx.__enter__()
                yA = ps.tile([65, 512], F32, tag="y", bufs=2,
                             name=f"yA{p}_{c}")
                yB = ps.tile([65, 512], F32, tag="y", bufs=2,
                             name=f"yB{p}_{c}")
                qtc = qT.pop((p, c))
                ctx.__exit__(None, None, None)
                for kt in range(nfull + 4):
                    ctx = nc.named_scope(f"at{c}_{p}"); ctx.__enter__()
                    di = kt - nfull
                    d = 0 if di < 0 else di * 128
                    s_ps = ps.tile([128, 1024], F32, tag="s", bufs=2,
                                   name=f"s{p}_{c}_{kt}")
                    ksl = kT[p][:, kt * 128:(kt + 1) * 128]
                    if di < 0:
                        # off-diagonal: full width, no mask
                        nc.tensor.matmul(s_ps[:, 0:512], ksl[0:64, :],
                                         qtc[0:64, :], start=True, stop=True,
                                         tile_position=(0, 0))
                        nc.tensor.matmul(s_ps[:, 512:1024], ksl[64:128, :],
                                         qtc[64:128, :], start=True, stop=True,
                                         tile_position=(64, 0))
                    else:
                        # diagonal k-tile: q < d is fully masked (skipped);
                        # [d,d+128) is the triangular block (mask add);
                        # [d+128,512) is unmasked.
                        if d + 128 < 512:
                            nc.tensor.matmul(
                                s_ps[:, d + 128:512], ksl[0:64, :],
                                qtc[0:64, d + 128:512], start=True, stop=True,
                                tile_position=(0, 0))
                            nc.tensor.matmul(
                                s_ps[:, 512 + d + 128:1024], ksl[64:128, :],
                                qtc[64:128, d + 128:512], start=True,
                                stop=True, tile_position=(64, 0))
                        nc.tensor.matmul(
                            s_ps[:, d:d + 128], ksl[0:64, :],
                            qtc[0:64, d:d + 128], start=True, stop=False,
                            tile_position=(0, 0))
                        nc.tensor.matmul(
                            s_ps[:, 512 + d:512 + d + 128], ksl[64:128, :],
                            qtc[64:128, d:d + 128], start=True, stop=False,
                            tile_position=(64, 0))
                        nc.tensor.matmul(s_ps[:, d:d + 128], ident, mask,
                                         start=False, stop=True)
                        nc.tensor.matmul(s_ps[:, 512 + d:512 + d + 128],
                                         ident, mask, start=False, stop=True)
                    pt = wk.tile([128, 1024], BF16, tag="P", bufs=3,
                                 name=f"P{p}_{c}_{kt}")
                    if d == 0:
                        nc.scalar.activation(pt, s_ps, EXP, scale=SCALE)
                    else:
                        nc.scalar.activation(pt[:, d:512], s_ps[:, d:512],
                                             EXP, scale=SCALE)
                        nc.scalar.activation(pt[:, 512 + d:1024],
                                             s_ps[:, 512 + d:1024],
                                             EXP, scale=SCALE)
                    nc.tensor.matmul(
                        yA[:, d:512], vt[:, kt, (2 * p) * 65:(2 * p) * 65 + 65],
                        pt[:, d:512],
                        start=(kt == 0), stop=(di == 3),
                        skip_group_check=(di >= 0))
                    nc.tensor.matmul(
                        yB[:, d:512],
                        vt[:, kt, (2 * p + 1) * 65:(2 * p + 1) * 65 + 65],
                        pt[:, 512 + d:1024],
                        start=(kt == 0), stop=(di == 3),
                        skip_group_check=(di >= 0))
                    ctx.__exit__(None, None, None)
                    yield
                # normalize unit: A/B chains emitted recip,recip /
                # bcast,bcast / mul,mul so DVE and GPSIMD overlap.
                ctx = nc.named_scope(f"at{c}_{p}"); ctx.__enter__()
                yt = wk.tile([128, 512], BF16, tag="yT", bufs=10,
                             name=f"yT{p}_{c}")
                rcs, rrs = [], []
                for h, yps in ((0, yA), (1, yB)):
                    drow = wk.tile([1, 512], F32, tag="dr", bufs=2,
                                   name=f"dr{p}_{c}_{h}")
                    nc.vector.tensor_copy(drow, yps[64:65, :])
                    rc = wk.tile([1, 512], F32, tag="rc", bufs=2,
                                 name=f"rc{p}_{c}_{h}")
                    nc.vector.reciprocal_approx_fast(rc, drow)
                    rcs.append(rc)
                for h in range(2):
                    rr = wk.tile([64, 512], F32, tag="rr", bufs=2,
                                 name=f"rr{p}_{c}_{h}")
                    nc.gpsimd.partition_broadcast(rr, rcs[h])
                    rrs.append(rr)
                for h, yps in ((0, yA), (1, yB)):
                    nc.vector.tensor_mul(yt[h * 64:(h + 1) * 64, :],
                                         yps[0:64, :], rrs[h])
                yT[(p, c)] = yt
                ctx.__exit__(None, None, None)
                yield

            def cproj_half(c, half):
                # transposed output layout: o^T[oc, t] — bias is a
                # per-partition scalar handled on ACT (same table as Exp),
                # yT needs no t4 slicing; the host transposes the result.
                # One unit = 2 matmuls.
                for ob in range(4 * half, 4 * half + 4):
                    o_ps = ps.tile([128, 512], F32, tag="pj", bufs=2,
                                   name=f"ops{c}_{ob}")
                    for pg in range(2):
                        ctx = nc.named_scope(f"cp{c}_{half}"); ctx.__enter__()
                        load_c_consts()
                        for p in (2 * pg, 2 * pg + 1):
                            nc.tensor.matmul(
                                o_ps, wp[:, p, ob, :], yT[(p, c)],
                                start=(p == 0), stop=(p == PAIRS - 1))
                        if pg == 1:
                            ot = wk.tile([128, 512], F32, tag="o", bufs=2,
                                         name=f"o{c}_{ob}")
                            nc.scalar.activation(
                                ot, o_ps,
                                mybir.ActivationFunctionType.Identity,
                                bias=bo[:, ob:ob + 1])
                            orow = out_d.ap()[ob * 128:(ob + 1) * 128, :]
                            if c == NCH - 1:
                                # tail: halve each transfer across two of
                                # the three queues so the drain overlaps
                                engs = (nc.sync, nc.gpsimd, nc.scalar)
                                e0 = engs[ob % 3]
                                e1 = engs[(ob + 1) % 3]
                                e0.dma_start(
                                    orow[:, c * CH:c * CH + 256],
                                    ot[:, 0:256])
                                e1.dma_start(
                                    orow[:, c * CH + 256:(c + 1) * CH],
                                    ot[:, 256:512])
                            else:
                                eng = (nc.sync if ob % 2 == 0
                                       else nc.gpsimd)
                                eng.dma_start(
                                    orow[:, c * CH:(c + 1) * CH], ot)
                        ctx.__exit__(None, None, None)
                        yield
                if half == 1:
                    for p in range(PAIRS):
                        yT.pop((p, c))

            # ---- unit-level scheduler ----
            # Stream A: attention pairs in order.  Stream B: projections +
            # c_proj.  Units are interleaved so the in-order PE queue always
            # holds exp-independent matmul work between ACT-dependent ones
            # (keeps the PE pstate ramped and hides the S->exp->PV latency).
            # Gates: attn(c,*) may not start before qk(c,*) and v(c) are
            # fully emitted (PE in-order would deadlock otherwise);
            # cproj(c,*) may not start before attn(c,3) is done.
            a_phases = [(c, p) for c in range(NCH) for p in range(PAIRS)]
            b_phases = ([("qk", 0, 0), ("qk", 0, 1), ("v", 0)] +
                        [("qk", 1, 0), ("qk", 1, 1), ("v", 1),
                         ("qk", 2, 0), ("qk", 2, 1), ("v", 2),
                         ("cp", 0, 0), ("cp", 0, 1),
                         ("qk", 3, 0), ("qk", 3, 1), ("v", 3),
                         ("cp", 1, 0), ("cp", 1, 1),
                         ("cp", 2, 0), ("cp", 2, 1),
                         ("cp", 3, 0), ("cp", 3, 1)])

            def b_units(ph):
                return {"qk": 16, "v": 16, "cp": 8}[ph[0]]

            def make_b(ph):
                kind = ph[0]
                if kind == "qk":
                    return proj_qk(ph[1], ph[2])
                if kind == "v":
                    return proj_v(ph[1])
                return cproj_half(ph[1], ph[2])

            b_done = set()    # finished b phases
            a_done = set()    # finished attn pairs
            rem_a = sum(4 * c + 5 for c, p in a_phases)
            b_left = [b_units(ph) for ph in b_phases]
            ai, bi = 0, 0
            a_gen = b_gen = None
            bal = 0.0

            def b_gate_ok(j):
                return (b_phases[j][0] != "cp"
                        or (b_phases[j][1], 3) in a_done)

            def rem_b_avail():
                # emittable-prefix B units: stop at the first gated phase so
                # reserved (gated) c_proj work doesn't get spent early
                tot = 0
                for j in range(bi, len(b_phases)):
                    if not b_gate_ok(j):
                        break
                    tot += b_left[j]
                return tot

            while True:
                can_a = a_gen is not None or (
                    ai < len(a_phases)
                    and ("v", a_phases[ai][0]) in b_done)
                can_b = b_gen is not None or (
                    bi < len(b_phases) and b_gate_ok(bi))
                if not can_a and not can_b:
                    if ai >= len(a_phases) and bi >= len(b_phases):
                        break
                    raise RuntimeError("scheduler deadlock")
                take_b = can_b and (not can_a or bal >= 1.0)
                if take_b:
                    if b_gen is None:
                        b_gen = make_b(b_phases[bi])
                    try:
                        next(b_gen)
                        b_left[bi] -= 1
                        if bal >= 1.0:
                            bal -= 1.0
                    except StopIteration:
                        b_done.add(b_phases[bi][:2])
                        b_done.add(b_phases[bi])
                        bi += 1
                        b_gen = None
                else:
                    if a_gen is None:
                        a_gen = attn_pair(*a_phases[ai])
                    try:
                        next(a_gen)
                        rem_a -= 1
                        bal += rem_b_avail() / max(rem_a, 1)
                    except StopIteration:
                        a_done.add(a_phases[ai])
                        ai += 1
                        a_gen = None

    nc.compile()
    return nc


_NC_CACHE = []


def _get_nc():
    if not _NC_CACHE:
        _NC_CACHE.append(build_nc())
    return _NC_CACHE[0]


def _host_consts():
    ident = np.eye(128, dtype=np.float32)
    kk = np.arange(128, dtype=np.int64)[:, None]
    jj = np.arange(128, dtype=np.int64)[None, :]
    tri = np.where(jj < kk, NEG, 0.0).astype(np.float32)
    return ident, tri


def _make_in_maps(x, W_attn, b_attn, W_proj, b_proj):
    ident, tri = _host_consts()
    in_maps = []
    for core in range(8):
        b, hg = core // 2, core % 2
        sl = slice(hg * 512, (hg + 1) * 512)
        w_q = W_attn[:, 0:1024][:, sl]
        w_k = W_attn[:, 1024:2048][:, sl]
        w_v = W_attn[:, 2048:3072][:, sl]
        in_maps.append({
            "x_t": np.ascontiguousarray(x[b].T).astype(BF16NP),
            "w_qk": np.ascontiguousarray(
                np.concatenate([w_q, w_k], axis=1).reshape(8, 128, 8, 128)
                .transpose(2, 1, 0, 3).reshape(1024, 1024)).astype(BF16NP),
            "w_v": np.ascontiguousarray(
                w_v.reshape(8, 128, 512).transpose(1, 0, 2)
                .reshape(128, 4096)).astype(BF16NP),
            "w_p": np.ascontiguousarray(
                W_proj[sl, :].reshape(4, 128, 8, 128).transpose(1, 0, 2, 3)
                .reshape(128, 4096)).astype(BF16NP),
            "b_qk": np.ascontiguousarray(
                np.concatenate([b_attn[0:1024][sl], b_attn[1024:2048][sl]])
                .reshape(8, 128).T),
            "b_v": np.ascontiguousarray(b_attn[2048:3072][sl]),
            "b_o": np.ascontiguousarray(
                (b_proj if hg == 0 else np.zeros_like(b_proj))
                .astype(np.float32).reshape(8, 128).T),
            "ident": ident.astype(BF16NP),
            "masks": tri.astype(BF16NP),
        })
    return in_maps


def _run(inputs, trace=False):
    x = np.asarray(inputs["x"], dtype=np.float32)
    W_attn = np.asarray(inputs["W_attn"], dtype=np.float32)
    b_attn = np.asarray(inputs["b_attn"], dtype=np.float32)
    W_proj = np.asarray(inputs["W_proj"], dtype=np.float32)
    b_proj = np.asarray(inputs["b_proj"], dtype=np.float32)

    nc = _get_nc()
    in_maps = _make_in_maps(x, W_attn, b_attn, W_proj, b_proj)
    res = run_bass_kernel_spmd(nc, in_maps, core_ids=list(range(8)),
                               trace=trace)
    out = np.empty((B, T, C), dtype=np.float32)
    for b in range(B):
        out[b] = (res.results[2 * b]["out_p"]
                  + res.results[2 * b + 1]["out_p"]).T
    return out, res


def kernel(**inputs) -> np.ndarray:
    out, _ = _run(inputs, trace=False)
    return out
